# revision 40
# baseline (speedup 1.0000x reference)
"""BitNet GQA attention layer on 8 TRN2 NeuronCores.

Sharding: token-parallel. B*S = 2048 tokens -> 256 per core (core c: batch
c//4, quarter c%4). Weights are split 8-way along the contraction dim for
quantization (exact global absmean via tiny AllReduces), then the ternary
integer weights are AllGathered in bf16 (three pipelined AGs: k+v first so
K/V projections start early, then q, then o). K/V are AllGathered within
each batch's 4-core group, hidden under Q-projection + Q-rope. All BitNet
matmuls run as exact integer arithmetic in bf16 (acts in [-128,127],
weights in {-1,0,1}) with fp32 PSUM accumulation; dequantization scales are
applied to the fp32 results.
"""

import sys

sys.path.insert(0, "/opt/trn_rl_repo")

import numpy as np
import ml_dtypes

import concourse.bass as bass
import concourse.mybir as mybir
import concourse.tile as tile
from concourse import bacc
from concourse import bass_utils
from concourse.masks import make_identity

F32 = mybir.dt.float32
BF16 = mybir.dt.bfloat16
FP8 = mybir.dt.float8e4
AX = mybir.AxisListType.X
OP = mybir.AluOpType
AF = mybir.ActivationFunctionType

B, S, H = 2, 1024, 2048
NH, NKV, HD = 16, 8, 128
NC = 8
T = (B * S) // NC  # 256 tokens per core
TB = T // 128  # 2 token tiles per core
HSL = H // NC  # 256 weight rows per core
EPS = 1e-6
RND = 12582912.0  # 1.5 * 2**23: fp32 add => round-to-nearest-even
INV_SQRT_HD = 1.0 / float(np.sqrt(HD))
KTILES = S // 128  # 8 key tiles per batch
GROUP = 4  # cores per batch

OQ, OK, OV, OO = H, NKV * HD, NKV * HD, H  # 2048, 1024, 1024, 2048
OW = {"q": OQ, "k": OK, "v": OV, "o": OO}
WNUMEL = {m: OW[m] * H for m in OW}
HI_N = H // 128  # 16 contraction tiles

_CACHE = {}


def _build():
    nc = bacc.Bacc("TRN2", target_bir_lowering=False, debug=False, num_devices=NC)

    x_sl = nc.dram_tensor("x_sl", [T, H], F32, kind="ExternalInput")
    cosq = nc.dram_tensor("cosq", [T, HD], F32, kind="ExternalInput")
    sinq = nc.dram_tensor("sinq", [T, HD], F32, kind="ExternalInput")
    cosk = nc.dram_tensor("cosk", [T, HD], F32, kind="ExternalInput")
    sink = nc.dram_tensor("sink", [T, HD], F32, kind="ExternalInput")
    w_sl = {
        "q": nc.dram_tensor("wq_sl", [HSL, OQ], F32, kind="ExternalInput"),
        "k": nc.dram_tensor("wk_sl", [HSL, OK], F32, kind="ExternalInput"),
        "v": nc.dram_tensor("wv_sl", [HSL, OV], F32, kind="ExternalInput"),
        "o": nc.dram_tensor("wo_sl", [HSL, OO], F32, kind="ExternalInput"),
    }
    mask_in = nc.dram_tensor("mask", [128, KTILES + TB, T], BF16, kind="ExternalInput")
    # cols 0-3: numel for k,v,q,o ; cols 4-7: 1/numel for k,v,q,o
    wconst = nc.dram_tensor("wconst", [1, 8], F32, kind="ExternalInput")
    out = nc.dram_tensor("out", [T, H], F32, kind="ExternalOutput")

    with tile.TileContext(nc) as tc:
        _build_body(nc, tc, x_sl, cosq, sinq, cosk, sink, w_sl, mask_in, wconst, out)

    nc.compile()
    return nc


def _build_body(nc, tc, x_sl, cosq, sinq, cosk, sink, w_sl, mask_in, wconst, out):
    sync = nc.sync

    with (
        tc.tile_pool(name="dram", bufs=1, space="DRAM") as dram,
        tc.tile_pool(name="const", bufs=1) as constp,
        tc.tile_pool(name="vecs", bufs=1) as vecs,
        tc.tile_pool(name="persist", bufs=1) as persist,
        tc.tile_pool(name="ptrans", bufs=2, space="PSUM") as ptrans,
    ):
        # ---- DRAM bounce buffers for collectives ----
        wag_k = dram.tile([HSL, OK], FP8)
        wint_k = dram.tile([H, OK], FP8, addr_space="Shared")
        wag_v = dram.tile([HSL, OV], FP8)
        wint_v = dram.tile([H, OV], FP8, addr_space="Shared")
        wag_q = dram.tile([HSL, OQ], FP8)
        wint_q = dram.tile([H, OQ], FP8, addr_space="Shared")
        wag_o = dram.tile([HSL, OO], FP8)
        wint_o = dram.tile([H, OO], FP8, addr_space="Shared")
        ar1_in = dram.tile([1, 8], F32)
        ar1_out = dram.tile([1, 8], F32, addr_space="Shared")
        ar2_in = dram.tile([1, 8], F32)
        ar2_out = dram.tile([1, 8], F32, addr_space="Shared")
        k_in = dram.tile([128, NKV * T], BF16)
        k_out = dram.tile([512, NKV * T], BF16)
        v_in = dram.tile([128, TB * OV], BF16)
        v_out = dram.tile([512, TB * OV], BF16)

        # warmup: absorb the collective engine's cold-start latency
        warm_in = dram.tile([1, 8], F32)
        warm_out = dram.tile([1, 8], F32, addr_space="Shared")
        wz = constp.tile([1, 8], F32)
        nc.vector.memset(wz, 0.0)
        nc.scalar.dma_start(warm_in, wz)
        nc.gpsimd.collective_compute(
            "AllReduce", OP.add, replica_groups=[list(range(NC))],
            ins=[warm_in.opt()], outs=[warm_out.opt()],
        )

        # ---- constants ----
        ident = constp.tile([128, 128], F32)
        make_identity(nc, ident)
        ones1 = constp.tile([1, 128], F32)
        nc.vector.memset(ones1, 1.0)
        onescol = constp.tile([128, 1], F32)
        nc.vector.memset(onescol, 1.0)
        wconst_sb = constp.tile([1, 8], F32)
        sync.dma_start(wconst_sb, wconst.ap())
        negrnd = constp.tile([128, 1], F32)
        nc.vector.memset(negrnd, -RND)
        epsb = constp.tile([128, 1], F32)
        nc.vector.memset(epsb, EPS)
        cs = {}
        for nm, t in (("cq", cosq), ("sq", sinq), ("ck", cosk), ("sk", sink)):
            c = constp.tile([128, TB, HD], F32, name=f"cs_{nm}")
            sync.dma_start(c, t.ap().rearrange("(a p) d -> p a d", p=128))
            cs[nm] = c
        # persistent activations
        xqT = persist.tile([128, HI_N, T], BF16)  # [h%128, h//128, t]

        # ====== Phase W: weight scales + quantize + pipelined allgathers ======
        def w_sums_group(mats, psmall, ar_in, ar_out, wraws, label):
            """Load slices of `mats`, abs-sum, kick the AllReduce."""
            wab = {}
            for m in mats:
                for pt in range(2):
                    wr = wraws[m].tile([128, OW[m]], F32, name=f"wr_{m}{pt}")
                    sync.dma_start(wr, w_sl[m].ap()[pt * 128 : (pt + 1) * 128, :])
                    wab[(m, pt)] = wr
            red0 = vecs.tile([128, 2], F32, name=f"red0_{label}")
            red1 = vecs.tile([128, 2], F32, name=f"red1_{label}")
            for mi, m in enumerate(mats):
                for pt, red in ((0, red0), (1, red1)):
                    nc.vector.tensor_reduce(
                        red[:, mi : mi + 1], wab[(m, pt)], axis=AX, op=OP.add,
                        apply_absolute_value=True,
                    )
            redc = vecs.tile([128, 2], F32, name=f"redc_{label}")
            nc.vector.tensor_add(redc, red0, red1)
            ps = psmall.tile([1, 2], F32, name=f"ps_{label}", tag="psm")
            nc.tensor.matmul(ps, onescol, redc, start=True, stop=True)
            sums = vecs.tile([1, 8], F32, name=f"sums_{label}")
            nc.vector.memset(sums, 0.0)
            nc.scalar.copy(sums[:, 0:2], ps)
            nc.scalar.dma_start(ar_in, sums)
            nc.gpsimd.collective_compute(
                "AllReduce", OP.add, replica_groups=[list(range(NC))],
                ins=[ar_in.opt()], outs=[ar_out.opt()],
            )
            return wab

        def w_scales_group(mats, psmall, ar_out, label):
            """Read back the AllReduce and build the [128,4] scale tile."""
            g = vecs.tile([1, 8], F32, name=f"g_{label}")
            nc.scalar.dma_start(g, ar_out)
            r2 = vecs.tile([1, 2], F32, name=f"r2_{label}")
            nc.vector.reciprocal(r2, g[:, 0:2])
            sw4 = vecs.tile([1, 4], F32, name=f"sw4_{label}")
            ncol = {("k", "v"): (0, 2), ("q", "o"): (2, 4)}[tuple(mats)]
            nc.vector.tensor_mul(sw4[:, 0:2], r2, wconst_sb[:, ncol[0] : ncol[1]])
            nc.vector.tensor_mul(
                sw4[:, 2:4], g[:, 0:2], wconst_sb[:, 4 + ncol[0] : 4 + ncol[1]]
            )
            pb = psmall.tile([128, 4], F32, name=f"pb_{label}", tag="psm")
            nc.tensor.matmul(pb, ones1, sw4, start=True, stop=True)
            sb = vecs.tile([128, 4], F32, name=f"sb_{label}")
            nc.scalar.copy(sb, pb)
            return sb

        def w_quant(wab, m, mi, sb, wtmp, wq8, dst, col0):
            for pt in range(2):
                wr = wab[(m, pt)]
                tmp = wtmp.tile([128, OW[m]], F32, tag="wtmp")
                nc.vector.tensor_scalar(
                    tmp, wr, sb[:, mi : mi + 1], RND, op0=OP.mult, op1=OP.add
                )
                nc.vector.tensor_scalar(
                    tmp, tmp, -RND, 1.0, op0=OP.add, op1=OP.min
                )
                wi = wq8.tile([128, OW[m]], FP8, tag="wi")
                nc.vector.tensor_scalar(wi, tmp, -1.0, None, op0=OP.max)
                nc.scalar.dma_start(
                    dst[pt * 128 : (pt + 1) * 128, col0 : col0 + OW[m]], wi
                )

        rswb = {}
        with (
            tc.tile_pool(name="wraw_q", bufs=1) as wraw_q,
            tc.tile_pool(name="wraw_k", bufs=1) as wraw_k,
            tc.tile_pool(name="wraw_v", bufs=1) as wraw_v,
            tc.tile_pool(name="wraw_o", bufs=1) as wraw_o,
            tc.tile_pool(name="wtmp", bufs=2) as wtmp,
            tc.tile_pool(name="wq8", bufs=2) as wq8,
            tc.tile_pool(name="psmall", bufs=2, space="PSUM") as psmall,
        ):
            wraws = {"q": wraw_q, "k": wraw_k, "v": wraw_v, "o": wraw_o}
            wab_kv = w_sums_group(("k", "v"), psmall, ar1_in, ar1_out, wraws, "kv")
            # x loads right behind wk/wv so the x pipeline fills the AR window
            xs_t = []
            with tc.tile_pool(name="xraw", bufs=2) as xraw:
                for tb in range(TB):
                    xs = xraw.tile([128, H], F32, tag="xs", name=f"xs{tb}")
                    sync.dma_start(xs, x_sl.ap()[tb * 128 : (tb + 1) * 128, :])
                    xs_t.append(xs)
                wab_qo = w_sums_group(
                    ("q", "o"), psmall, ar2_in, ar2_out, wraws, "qo"
                )

                # ====== Phase X: act_quant(x) + transpose ======
                dqx = []
                for tb in range(TB):
                    xs = xs_t[tb]
                    axm = vecs.tile([128, 1], F32, name=f"axm{tb}")
                    nc.vector.tensor_reduce(
                        axm, xs, axis=AX, op=OP.max, apply_absolute_value=True
                    )
                    rsx = vecs.tile([128, 1], F32, name=f"rsx{tb}")
                    nc.vector.reciprocal(rsx, axm)
                    sxq = vecs.tile([128, 1], F32, name=f"sxq{tb}")
                    nc.vector.tensor_scalar_mul(sxq, rsx, 127.0)
                    dq = vecs.tile([128, 1], F32, name=f"dqx{tb}")
                    nc.vector.tensor_scalar_mul(dq, axm, 1.0 / 127.0)
                    dqx.append(dq)
                    nc.vector.tensor_scalar(
                        xs, xs, sxq, RND, op0=OP.mult, op1=OP.add
                    )
                    for hg in range(0, HI_N, 4):
                        pt4 = ptrans.tile([128, 4, 128], F32, tag="ptr")
                        for i in range(4):
                            hi = hg + i
                            nc.tensor.transpose(
                                pt4[:, i, :], xs[:, hi * 128 : (hi + 1) * 128], ident
                            )
                        nc.scalar.activation(
                            xqT[:, hg : hg + 4, tb * 128 : (tb + 1) * 128],
                            pt4, AF.Identity, bias=negrnd,
                        )

            sb_kv = w_scales_group(("k", "v"), psmall, ar1_out, "kv")
            w_quant(wab_kv, "k", 0, sb_kv, wtmp, wq8, wag_k, 0)
            nc.gpsimd.collective_compute(
                "AllGather", OP.bypass, replica_groups=[list(range(NC))],
                ins=[wag_k.opt()], outs=[wint_k.opt()],
            )
            w_quant(wab_kv, "v", 1, sb_kv, wtmp, wq8, wag_v, 0)
            nc.gpsimd.collective_compute(
                "AllGather", OP.bypass, replica_groups=[list(range(NC))],
                ins=[wag_v.opt()], outs=[wint_v.opt()],
            )
            rswb["k"] = sb_kv[:, 2:3]
            rswb["v"] = sb_kv[:, 3:4]
            sb_qo = w_scales_group(("q", "o"), psmall, ar2_out, "qo")
            w_quant(wab_qo, "q", 0, sb_qo, wtmp, wq8, wag_q, 0)
            nc.gpsimd.collective_compute(
                "AllGather", OP.bypass, replica_groups=[list(range(NC))],
                ins=[wag_q.opt()], outs=[wint_q.opt()],
            )
            w_quant(wab_qo, "o", 1, sb_qo, wtmp, wq8, wag_o, 0)
            rswb["q"] = sb_qo[:, 2:3]
            rswb["o"] = sb_qo[:, 3:4]

        # dequant vectors (absmax/127 * 1/s_w)
        dqv = {}
        for m in ("q", "k", "v", "o"):
            for tb in range(TB):
                d = vecs.tile([128, 1], F32, name=f"dqv_{m}{tb}")
                nc.vector.tensor_mul(d, dqx[tb], rswb[m])
                dqv[(m, tb)] = d

        q_sb = persist.tile([128, TB, OQ], F32, tag="qsb")
        k_sb = persist.tile([128, TB, OK], F32)
        v_loc = persist.tile([128, TB, NKV, 130], BF16)
        nc.vector.memset(v_loc, 1.0)
        qT = persist.tile([128, NH, T], BF16)  # [d, head, t]
        kT = persist.tile([128, NKV, T], BF16, tag="t8", padded_shape=[128, HI_N, T])

        def proj_load(wint_src, o_w, m, wpool):
            """Stream the fp8 weight matrix in 4 hi-chunks (pipelined)."""
            src3 = wint_src.rearrange("(hi p) o -> p hi o", p=128)
            chunks = []
            for cg in range(4):
                wst = wpool.tile(
                    [128, 4, o_w], FP8, tag="wst",
                    padded_shape=[128, 4, OQ], name=f"wst_{m}{cg}",
                )
                sync.dma_start(wst, src3[:, cg * 4 : (cg + 1) * 4, :])
                chunks.append(wst)
            return chunks

        def proj_tb(chunks, col0, o_w, m, tb, dst_fn, ppool):
            """dequant(xqT.T @ w_int) for one token tile over all o-chunks."""
            for oc in range(o_w // 512):
                pp = ppool.tile([128, 512], F32, tag="pp")
                for hi in range(HI_N):
                    nc.tensor.matmul(
                        pp,
                        xqT[:, hi, tb * 128 : (tb + 1) * 128],
                        chunks[hi // 4][
                            :, hi % 4, col0 + oc * 512 : col0 + (oc + 1) * 512
                        ],
                        start=(hi == 0),
                        stop=(hi == HI_N - 1),
                    )
                nc.vector.tensor_scalar(
                    dst_fn(tb, oc), pp, dqv[(m, tb)], None, op0=OP.mult
                )

        def rope_batch(src_sb, tb, nh, cosn, sinn, dstT, ropep, label):
            w = nh * 128
            blk = src_sb[:, tb, :]  # [128, w] f32
            sq = ropep.tile([128, w], F32, tag="unf", padded_shape=[128, NH * 128])
            nc.scalar.activation(sq, blk, AF.Square)
            ms = vecs.tile([128, nh], F32, name=f"ms_{label}{tb}")
            nc.vector.tensor_reduce(
                ms, sq.rearrange("p (h d) -> p h d", h=nh), axis=AX, op=OP.add
            )
            rms = vecs.tile([128, nh], F32, name=f"rms_{label}{tb}")
            nc.scalar.activation(rms, ms, AF.Sqrt, scale=1.0 / HD, bias=epsb)
            rn = vecs.tile([128, nh], F32, name=f"rn_{label}{tb}")
            nc.vector.reciprocal(rn, rms)
            rnb = rn.to_broadcast([128, nh, 128])
            blk3 = blk.rearrange("p (h d) -> p h d", h=nh)
            un2 = ropep.tile(
                [128, nh * 128], F32, tag="unf", padded_shape=[128, NH * 128],
                name="un2",
            )
            un = un2.rearrange("p (h d) -> p h d", h=nh)
            nc.vector.tensor_mul(un, blk3, rnb)
            cosb = (
                cs[cosn][:, tb, :]
                .rearrange("p (one d) -> p one d", one=1)
                .to_broadcast([128, nh, 128])
            )
            sinb = (
                cs[sinn][:, tb, :]
                .rearrange("p (one d) -> p one d", one=1)
                .to_broadcast([128, nh, 128])
            )
            ra = ropep.tile([128, nh, 128], F32, tag="ra", padded_shape=[128, NH, 128])
            nc.vector.tensor_mul(ra, un, cosb)
            rb = ropep.tile([128, nh, 128], F32, tag="rb", padded_shape=[128, NH, 128])
            nc.vector.tensor_mul(rb[:, :, 0:64], un[:, :, 64:128], sinb[:, :, 0:64])
            nc.vector.tensor_mul(rb[:, :, 64:128], un[:, :, 0:64], sinb[:, :, 64:128])
            nc.vector.tensor_add(ra, ra, rb)
            for hg in range(0, nh, 4):
                pt4 = ptrans.tile([128, 4, 128], F32, tag="ptr")
                for i in range(4):
                    nc.tensor.transpose(pt4[:, i, :], ra[:, hg + i, :], ident)
                nc.scalar.activation(
                    dstT[:, hg : hg + 4, tb * 128 : (tb + 1) * 128], pt4, AF.Copy
                )

        # ====== K/V projections + K rope + KV allgather ======
        with (
            tc.tile_pool(name="wmm1", bufs=8) as wmm1,
            tc.tile_pool(name="pproj1", bufs=3, space="PSUM") as pproj1,
            tc.tile_pool(name="ropek", bufs=1) as ropek,
        ):
            wst_k = proj_load(wint_k, OK, "k", wmm1)
            for tb in range(TB):
                proj_tb(wst_k, 0, OK, "k", tb,
                        lambda tb, oc: k_sb[:, tb, oc * 512 : (oc + 1) * 512],
                        pproj1)
                rope_batch(k_sb, tb, NKV, "ck", "sk", kT, ropek, "k")
            sync.dma_start(
                k_in.rearrange("p (hk t) -> p hk t", hk=NKV), kT
            )
            nc.gpsimd.collective_compute(
                "AllGather", OP.bypass,
                replica_groups=[[0, 1, 2, 3], [4, 5, 6, 7]],
                ins=[k_in.opt()], outs=[k_out.opt()],
            )
            wst_v = proj_load(wint_v, OV, "v", wmm1)
            for tb in range(TB):
                proj_tb(wst_v, 0, OV, "v", tb,
                        lambda tb, oc: v_loc[:, tb, oc * 4 : (oc + 1) * 4, 0:128],
                        pproj1)
            sync.dma_start(
                v_in.rearrange("p (a hk d) -> p a hk d", a=TB, hk=NKV),
                v_loc[:, :, :, 0:128],
            )
            nc.gpsimd.collective_compute(
                "AllGather", OP.bypass,
                replica_groups=[[0, 1, 2, 3], [4, 5, 6, 7]],
                ins=[v_in.opt()], outs=[v_out.opt()],
            )

            # ====== Q projection + Q rope (overlap the KV allgathers) ======
            wst_q = proj_load(wint_q, OQ, "q", wmm1)
            for tb in range(TB):
                proj_tb(wst_q, 0, OQ, "q", tb,
                        lambda tb, oc: q_sb[:, tb, oc * 512 : (oc + 1) * 512],
                        pproj1)
                rope_batch(q_sb, tb, NH, "cq", "sq", qT, ropek, "q")
            nc.gpsimd.collective_compute(
                "AllGather", OP.bypass, replica_groups=[list(range(NC))],
                ins=[wag_o.opt()], outs=[wint_o.opt()],
            )

        mask_sb = persist.tile([128, KTILES + TB, T], BF16)
        sync.dma_start(mask_sb, mask_in.ap())
        attn = persist.tile([128, TB, H], F32, tag="qsb")  # reuse q_sb slot
        attn_loc = persist.tile([128, TB, NH, 132], F32)

        # ====== attention ======
        with (
            tc.tile_pool(name="pscore", bufs=2, space="PSUM") as pscore,
            tc.tile_pool(name="ppv", bufs=2, space="PSUM") as ppv,
            tc.tile_pool(name="pexp", bufs=4) as pexp,
        ):
            # local part: own K/V tiles (diagonal blocks) - no collective dep
            for h in range(NH):
                hk = h // 2
                pel = pexp.tile([128, TB, T], BF16, tag="pel")
                st = pscore.tile([128, 2, T], F32, tag="st")
                for a in range(TB):
                    nc.tensor.matmul(
                        st[:, a, :], kT[:, hk, a * 128 : (a + 1) * 128], qT[:, h, :],
                        start=True, stop=True,
                    )
                nc.scalar.activation(pel, st, AF.Exp, scale=INV_SQRT_HD)
                nc.vector.tensor_mul(pel, pel, mask_sb[:, KTILES : KTILES + TB, :])
                for tb in range(TB):
                    po = ppv.tile([128, 132], F32, tag="po", padded_shape=[128, 132])
                    for a in range(TB):
                        nc.tensor.matmul(
                            po[:, 0:129],
                            pel[:, a, tb * 128 : (tb + 1) * 128],
                            v_loc[:, a, hk, 0:129],
                            start=(a == 0),
                            stop=(a == TB - 1),
                        )
                    nc.vector.tensor_copy(attn_loc[:, tb, h, 0:129], po[:, 0:129])

            # gather readback
            kT_all = persist.tile([128, NKV, KTILES, 128], BF16)
            v_all = persist.tile([128, KTILES, NKV, 130], BF16)
            nc.vector.memset(v_all, 1.0)
            for cb in range(GROUP):
                # kT part: k_out row = 128*cb + d ; col = hk*256 + a*128 + t
                src_k = k_out[cb * 128 : (cb + 1) * 128, :].rearrange(
                    "d (hk t) -> d hk t", hk=NKV
                )
                sync.dma_start(kT_all[:, :, 2 * cb : 2 * cb + 2, :], src_k)
            for cb in range(GROUP):
                # v part: v_out row = 128*cb + p ; col = a*1024 + hk*128 + d
                src_v = v_out[cb * 128 : (cb + 1) * 128, :].rearrange(
                    "p (a hk d) -> p a hk d", a=TB, hk=NKV
                )
                sync.dma_start(v_all[:, 2 * cb : 2 * cb + 2, :, 0:128], src_v)

            # remote part: strictly-below-diagonal tiles from the allgather
            for h in range(NH):
                hk = h // 2
                pe = pexp.tile([128, KTILES, T], BF16, tag="pe")
                for jp in range(KTILES // 2):
                    st = pscore.tile([128, 2, T], F32, tag="st")
                    for i in range(2):
                        nc.tensor.matmul(
                            st[:, i, :], kT_all[:, hk, 2 * jp + i, :], qT[:, h, :],
                            start=True, stop=True,
                        )
                    nc.scalar.activation(
                        pe[:, 2 * jp : 2 * jp + 2, :], st, AF.Exp, scale=INV_SQRT_HD
                    )
                nc.vector.tensor_mul(pe, pe, mask_sb[:, 0:KTILES, :])
                for tb in range(TB):
                    po = ppv.tile([128, 132], F32, tag="po", padded_shape=[128, 132])
                    for j in range(KTILES):
                        nc.tensor.matmul(
                            po[:, 0:129],
                            pe[:, j, tb * 128 : (tb + 1) * 128],
                            v_all[:, j, hk, 0:129],
                            start=(j == 0),
                            stop=(j == KTILES - 1),
                        )
                    cmb = pexp.tile([128, 132], F32, tag="cmb")
                    nc.vector.tensor_add(
                        cmb[:, 0:129], po[:, 0:129], attn_loc[:, tb, h, 0:129]
                    )
                    rden = vecs.tile([128, 1], F32, name=f"rden{h}_{tb}")
                    nc.vector.reciprocal(rden, cmb[:, 128:129])
                    nc.vector.tensor_scalar(
                        attn[:, tb, h * 128 : (h + 1) * 128],
                        cmb[:, 0:128], rden, None, op0=OP.mult,
                    )

        # ====== act_quant(attn) + o_proj ======
        with (
            tc.tile_pool(name="oq", bufs=2) as oq,
            tc.tile_pool(name="wmm2", bufs=1) as wmm2,
            tc.tile_pool(name="pproj2", bufs=3, space="PSUM") as pproj2,
            tc.tile_pool(name="osb", bufs=2) as osb,
        ):
            aT = persist.tile([128, HI_N, T], BF16, tag="t8")
            dqo = []
            for tb in range(TB):
                axm = vecs.tile([128, 1], F32, name=f"oaxm{tb}")
                nc.vector.tensor_reduce(
                    axm, attn[:, tb, :], axis=AX, op=OP.max,
                    apply_absolute_value=True,
                )
                rsx = vecs.tile([128, 1], F32, name=f"orsx{tb}")
                nc.vector.reciprocal(rsx, axm)
                sxq = vecs.tile([128, 1], F32, name=f"osxq{tb}")
                nc.vector.tensor_scalar_mul(sxq, rsx, 127.0)
                dq = vecs.tile([128, 1], F32, name=f"odqx{tb}")
                nc.vector.tensor_scalar_mul(dq, axm, 1.0 / 127.0)
                d2 = vecs.tile([128, 1], F32, name=f"odq2{tb}")
                nc.vector.tensor_mul(d2, dq, rswb["o"])
                dqo.append(d2)
                ar = oq.tile([128, H], F32, tag="ar")
                nc.vector.tensor_scalar(
                    ar, attn[:, tb, :], sxq, RND, op0=OP.mult, op1=OP.add
                )
                for hg in range(0, HI_N, 4):
                    pt4 = ptrans.tile([128, 4, 128], F32, tag="ptr")
                    for i in range(4):
                        hi = hg + i
                        nc.tensor.transpose(
                            pt4[:, i, :], ar[:, hi * 128 : (hi + 1) * 128], ident
                        )
                    nc.scalar.activation(
                        aT[:, hg : hg + 4, tb * 128 : (tb + 1) * 128],
                        pt4, AF.Identity, bias=negrnd,
                    )

            src3 = wint_o.rearrange("(hi p) o -> p hi o", p=128)
            wsto = wmm2.tile([128, HI_N, OO], FP8, tag="wst2")
            sync.dma_start(wsto, src3)
            for tb in range(TB):
                for oc in range(OO // 512):
                    pp = pproj2.tile([128, 512], F32, tag="pp2")
                    for hi in range(HI_N):
                        nc.tensor.matmul(
                            pp,
                            aT[:, hi, tb * 128 : (tb + 1) * 128],
                            wsto[:, hi, oc * 512 : (oc + 1) * 512],
                            start=(hi == 0),
                            stop=(hi == HI_N - 1),
                        )
                    ot = osb.tile([128, 512], F32, tag="ot")
                    nc.vector.tensor_scalar(ot, pp, dqo[tb], None, op0=OP.mult)
                    sync.dma_start(
                        out.ap()[
                            tb * 128 : (tb + 1) * 128, oc * 512 : (oc + 1) * 512
                        ],
                        ot,
                    )


def _host_inputs(x, cos, sin, wq, wk, wv, wo, qn, kn):
    """Build the 8 per-core input maps (pure slicing / layout transforms)."""
    x2 = np.asarray(x, np.float32).reshape(B * S, H)
    cos = np.asarray(cos, np.float32)
    sin = np.asarray(sin, np.float32)
    qn = np.asarray(qn, np.float32)
    kn = np.asarray(kn, np.float32)
    # fold qk-norm weights into rope tables (exact identity when qn=kn=1)
    qn_rot = np.concatenate([qn[HD // 2 :], qn[: HD // 2]])
    kn_rot = np.concatenate([kn[HD // 2 :], kn[: HD // 2]])
    sgn = np.concatenate(
        [-np.ones(HD // 2, np.float32), np.ones(HD // 2, np.float32)]
    )
    cosq_t = cos * qn[None, :]
    sinq_t = sin * (qn_rot * sgn)[None, :]
    cosk_t = cos * kn[None, :]
    sink_t = sin * (kn_rot * sgn)[None, :]

    wt = {
        "q": np.asarray(wq, np.float32).T,  # [H, OQ]
        "k": np.asarray(wk, np.float32).T,
        "v": np.asarray(wv, np.float32).T,
        "o": np.asarray(wo, np.float32).T,  # [H(=in), OO]
    }
    worder = ("k", "v", "q", "o")
    wconst = np.concatenate(
        [
            np.array([WNUMEL[m] for m in worder], np.float32),
            np.array([1.0 / WNUMEL[m] for m in worder], np.float32),
        ]
    ).reshape(1, 8)

    in_maps = []
    for c in range(NC):
        qt = c % GROUP
        t0 = qt * T
        # strict mask [p, j, f]: key (128j+p) fully below this core's window
        p = np.arange(128)[:, None, None]
        j = np.arange(KTILES)[None, :, None]
        f = np.arange(T)[None, None, :]
        strict = ((128 * j + p) < t0) & (f >= 0)
        # diagonal masks for the two local key tiles
        a = np.arange(TB)[None, :, None]
        diag = (128 * a + p) <= f
        mask = np.concatenate([strict, diag], axis=1).astype(ml_dtypes.bfloat16)
        m = {
            "x_sl": np.ascontiguousarray(x2[c * T : (c + 1) * T]),
            "cosq": np.ascontiguousarray(cosq_t[t0 : t0 + T]),
            "sinq": np.ascontiguousarray(sinq_t[t0 : t0 + T]),
            "cosk": np.ascontiguousarray(cosk_t[t0 : t0 + T]),
            "sink": np.ascontiguousarray(sink_t[t0 : t0 + T]),
            "wq_sl": np.ascontiguousarray(wt["q"][c * HSL : (c + 1) * HSL]),
            "wk_sl": np.ascontiguousarray(wt["k"][c * HSL : (c + 1) * HSL]),
            "wv_sl": np.ascontiguousarray(wt["v"][c * HSL : (c + 1) * HSL]),
            "wo_sl": np.ascontiguousarray(wt["o"][c * HSL : (c + 1) * HSL]),
            "mask": mask,
            "wconst": wconst,
        }
        in_maps.append(m)
    return in_maps


def kernel(x, cos, sin, wq, wk, wv, wo, qn, kn):
    if "nc" not in _CACHE:
        _CACHE["nc"] = _build()
    nc = _CACHE["nc"]
    in_maps = _host_inputs(x, cos, sin, wq, wk, wv, wo, qn, kn)
    res = bass_utils.run_bass_kernel_spmd(nc, in_maps, core_ids=list(range(NC)))
    outs = [np.asarray(res.results[c]["out"]) for c in range(NC)]
    return np.concatenate(outs, axis=0).reshape(B, S, H).astype(np.float32)


# revision 44
# speedup vs baseline: 1.0082x; 1.0082x over previous
"""BitNet GQA attention layer on 8 TRN2 NeuronCores.

Sharding: token-parallel. B*S = 2048 tokens -> 256 per core (core c: batch
c//4, quarter c%4). Weights are split 8-way along the contraction dim for
quantization (exact global absmean via tiny AllReduces), then the ternary
integer weights are AllGathered in bf16 (three pipelined AGs: k+v first so
K/V projections start early, then q, then o). K/V are AllGathered within
each batch's 4-core group, hidden under Q-projection + Q-rope. All BitNet
matmuls run as exact integer arithmetic in bf16 (acts in [-128,127],
weights in {-1,0,1}) with fp32 PSUM accumulation; dequantization scales are
applied to the fp32 results.
"""

import sys

sys.path.insert(0, "/opt/trn_rl_repo")

import numpy as np
import ml_dtypes

import concourse.bass as bass
import concourse.mybir as mybir
import concourse.tile as tile
from concourse import bacc
from concourse import bass_utils
from concourse.masks import make_identity

F32 = mybir.dt.float32
BF16 = mybir.dt.bfloat16
FP8 = mybir.dt.float8e4
AX = mybir.AxisListType.X
OP = mybir.AluOpType
AF = mybir.ActivationFunctionType

B, S, H = 2, 1024, 2048
NH, NKV, HD = 16, 8, 128
NC = 8
T = (B * S) // NC  # 256 tokens per core
TB = T // 128  # 2 token tiles per core
HSL = H // NC  # 256 weight rows per core
EPS = 1e-6
RND = 12582912.0  # 1.5 * 2**23: fp32 add => round-to-nearest-even
INV_SQRT_HD = 1.0 / float(np.sqrt(HD))
KTILES = S // 128  # 8 key tiles per batch
GROUP = 4  # cores per batch

OQ, OK, OV, OO = H, NKV * HD, NKV * HD, H  # 2048, 1024, 1024, 2048
OW = {"q": OQ, "k": OK, "v": OV, "o": OO}
WNUMEL = {m: OW[m] * H for m in OW}
HI_N = H // 128  # 16 contraction tiles

_CACHE = {}


def _build():
    nc = bacc.Bacc("TRN2", target_bir_lowering=False, debug=False, num_devices=NC)

    x_sl = nc.dram_tensor("x_sl", [T, H], F32, kind="ExternalInput")
    cosq = nc.dram_tensor("cosq", [T, HD], F32, kind="ExternalInput")
    sinq = nc.dram_tensor("sinq", [T, HD], F32, kind="ExternalInput")
    cosk = nc.dram_tensor("cosk", [T, HD], F32, kind="ExternalInput")
    sink = nc.dram_tensor("sink", [T, HD], F32, kind="ExternalInput")
    w_sl = {
        "q": nc.dram_tensor("wq_sl", [HSL, OQ], F32, kind="ExternalInput"),
        "k": nc.dram_tensor("wk_sl", [HSL, OK], F32, kind="ExternalInput"),
        "v": nc.dram_tensor("wv_sl", [HSL, OV], F32, kind="ExternalInput"),
        "o": nc.dram_tensor("wo_sl", [HSL, OO], F32, kind="ExternalInput"),
    }
    mask_in = nc.dram_tensor("mask", [128, KTILES + TB, T], BF16, kind="ExternalInput")
    # cols 0-3: numel for k,v,q,o ; cols 4-7: 1/numel for k,v,q,o
    wconst = nc.dram_tensor("wconst", [1, 8], F32, kind="ExternalInput")
    out = nc.dram_tensor("out", [T, H], F32, kind="ExternalOutput")

    with tile.TileContext(nc) as tc:
        _build_body(nc, tc, x_sl, cosq, sinq, cosk, sink, w_sl, mask_in, wconst, out)

    nc.compile()
    return nc


def _build_body(nc, tc, x_sl, cosq, sinq, cosk, sink, w_sl, mask_in, wconst, out):
    sync = nc.sync

    with (
        tc.tile_pool(name="dram", bufs=1, space="DRAM") as dram,
        tc.tile_pool(name="const", bufs=1) as constp,
        tc.tile_pool(name="vecs", bufs=1) as vecs,
        tc.tile_pool(name="persist", bufs=1) as persist,
        tc.tile_pool(name="ptrans", bufs=2, space="PSUM") as ptrans,
    ):
        # ---- DRAM bounce buffers for collectives ----
        wag_kv = dram.tile([HSL, OK + OV], FP8)
        wint_kv = dram.tile([H, OK + OV], FP8, addr_space="Shared")
        wag_q = dram.tile([HSL, OQ], FP8)
        wint_q = dram.tile([H, OQ], FP8, addr_space="Shared")
        wag_o = dram.tile([HSL, OO], FP8)
        wint_o = dram.tile([H, OO], FP8, addr_space="Shared")
        ar1_in = dram.tile([1, 8], F32)
        ar1_out = dram.tile([1, 8], F32, addr_space="Shared")
        ar2_in = dram.tile([1, 8], F32)
        ar2_out = dram.tile([1, 8], F32, addr_space="Shared")
        k_in = dram.tile([128, NKV * T], BF16)
        k_out = dram.tile([512, NKV * T], BF16)
        v_in = dram.tile([128, TB * OV], BF16)
        v_out = dram.tile([512, TB * OV], BF16)

        # warmup: absorb the collective engine's cold-start latency
        warm_in = dram.tile([1, 8], F32)
        warm_out = dram.tile([1, 8], F32, addr_space="Shared")
        wz = constp.tile([1, 8], F32)
        nc.vector.memset(wz, 0.0)
        nc.scalar.dma_start(warm_in, wz)
        nc.gpsimd.collective_compute(
            "AllReduce", OP.add, replica_groups=[list(range(NC))],
            ins=[warm_in.opt()], outs=[warm_out.opt()],
        )

        # ---- constants ----
        ident = constp.tile([128, 128], F32)
        make_identity(nc, ident)
        ones1 = constp.tile([1, 128], F32)
        nc.vector.memset(ones1, 1.0)
        onescol = constp.tile([128, 1], F32)
        nc.vector.memset(onescol, 1.0)
        wconst_sb = constp.tile([1, 8], F32)
        sync.dma_start(wconst_sb, wconst.ap())
        negrnd = constp.tile([128, 1], F32)
        nc.vector.memset(negrnd, -RND)
        epsb = constp.tile([128, 1], F32)
        nc.vector.memset(epsb, EPS)
        cs = {}
        for nm, t in (("cq", cosq), ("sq", sinq), ("ck", cosk), ("sk", sink)):
            c = constp.tile([128, TB, HD], F32, name=f"cs_{nm}")
            sync.dma_start(c, t.ap().rearrange("(a p) d -> p a d", p=128))
            cs[nm] = c
        # persistent activations
        xqT = persist.tile([128, HI_N, T], BF16)  # [h%128, h//128, t]

        # ====== Phase W: weight scales + quantize + pipelined allgathers ======
        def w_sums_group(mats, psmall, ar_in, ar_out, wraws, label):
            """Load slices of `mats`, abs-sum, kick the AllReduce."""
            wab = {}
            for m in mats:
                for pt in range(2):
                    wr = wraws[m].tile([128, OW[m]], F32, name=f"wr_{m}{pt}")
                    sync.dma_start(wr, w_sl[m].ap()[pt * 128 : (pt + 1) * 128, :])
                    wab[(m, pt)] = wr
            red0 = vecs.tile([128, 2], F32, name=f"red0_{label}")
            red1 = vecs.tile([128, 2], F32, name=f"red1_{label}")
            for mi, m in enumerate(mats):
                for pt, red in ((0, red0), (1, red1)):
                    nc.vector.tensor_reduce(
                        red[:, mi : mi + 1], wab[(m, pt)], axis=AX, op=OP.add,
                        apply_absolute_value=True,
                    )
            redc = vecs.tile([128, 2], F32, name=f"redc_{label}")
            nc.vector.tensor_add(redc, red0, red1)
            ps = psmall.tile([1, 2], F32, name=f"ps_{label}", tag="psm")
            nc.tensor.matmul(ps, onescol, redc, start=True, stop=True)
            sums = vecs.tile([1, 8], F32, name=f"sums_{label}")
            nc.vector.memset(sums, 0.0)
            nc.scalar.copy(sums[:, 0:2], ps)
            nc.scalar.dma_start(ar_in, sums)
            nc.gpsimd.collective_compute(
                "AllReduce", OP.add, replica_groups=[list(range(NC))],
                ins=[ar_in.opt()], outs=[ar_out.opt()],
            )
            return wab

        def w_scales_group(mats, psmall, ar_out, label):
            """Read back the AllReduce and build the [128,4] scale tile."""
            g = vecs.tile([1, 8], F32, name=f"g_{label}")
            nc.scalar.dma_start(g, ar_out)
            r2 = vecs.tile([1, 2], F32, name=f"r2_{label}")
            nc.vector.reciprocal(r2, g[:, 0:2])
            sw4 = vecs.tile([1, 4], F32, name=f"sw4_{label}")
            ncol = {("k", "v"): (0, 2), ("q", "o"): (2, 4)}[tuple(mats)]
            nc.vector.tensor_mul(sw4[:, 0:2], r2, wconst_sb[:, ncol[0] : ncol[1]])
            nc.vector.tensor_mul(
                sw4[:, 2:4], g[:, 0:2], wconst_sb[:, 4 + ncol[0] : 4 + ncol[1]]
            )
            pb = psmall.tile([128, 4], F32, name=f"pb_{label}", tag="psm")
            nc.tensor.matmul(pb, ones1, sw4, start=True, stop=True)
            sb = vecs.tile([128, 4], F32, name=f"sb_{label}")
            nc.scalar.copy(sb, pb)
            return sb

        def w_quant(wab, m, mi, sb, wtmp, wq8, dst, col0):
            for pt in range(2):
                wr = wab[(m, pt)]
                tmp = wtmp.tile([128, OW[m]], F32, tag="wtmp")
                nc.vector.tensor_scalar(
                    tmp, wr, sb[:, mi : mi + 1], RND, op0=OP.mult, op1=OP.add
                )
                nc.vector.tensor_scalar(
                    tmp, tmp, -RND, 1.0, op0=OP.add, op1=OP.min
                )
                wi = wq8.tile([128, OW[m]], FP8, tag="wi")
                nc.vector.tensor_scalar(wi, tmp, -1.0, None, op0=OP.max)
                nc.scalar.dma_start(
                    dst[pt * 128 : (pt + 1) * 128, col0 : col0 + OW[m]], wi
                )

        rswb = {}
        with (
            tc.tile_pool(name="wraw_q", bufs=1) as wraw_q,
            tc.tile_pool(name="wraw_k", bufs=1) as wraw_k,
            tc.tile_pool(name="wraw_v", bufs=1) as wraw_v,
            tc.tile_pool(name="wraw_o", bufs=1) as wraw_o,
            tc.tile_pool(name="wtmp", bufs=2) as wtmp,
            tc.tile_pool(name="wq8", bufs=2) as wq8,
            tc.tile_pool(name="psmall", bufs=2, space="PSUM") as psmall,
        ):
            wraws = {"q": wraw_q, "k": wraw_k, "v": wraw_v, "o": wraw_o}
            wab_kv = w_sums_group(("k", "v"), psmall, ar1_in, ar1_out, wraws, "kv")
            # x loads right behind wk/wv so the x pipeline fills the AR window
            xs_t = []
            with tc.tile_pool(name="xraw", bufs=2) as xraw:
                for tb in range(TB):
                    xs = xraw.tile([128, H], F32, tag="xs", name=f"xs{tb}")
                    sync.dma_start(xs, x_sl.ap()[tb * 128 : (tb + 1) * 128, :])
                    xs_t.append(xs)
                wab_qo = w_sums_group(
                    ("q", "o"), psmall, ar2_in, ar2_out, wraws, "qo"
                )

                # ====== Phase X: act_quant(x) + transpose ======
                dqx = []
                for tb in range(TB):
                    xs = xs_t[tb]
                    axm = vecs.tile([128, 1], F32, name=f"axm{tb}")
                    nc.vector.tensor_reduce(
                        axm, xs, axis=AX, op=OP.max, apply_absolute_value=True
                    )
                    rsx = vecs.tile([128, 1], F32, name=f"rsx{tb}")
                    nc.vector.reciprocal(rsx, axm)
                    sxq = vecs.tile([128, 1], F32, name=f"sxq{tb}")
                    nc.vector.tensor_scalar_mul(sxq, rsx, 127.0)
                    dq = vecs.tile([128, 1], F32, name=f"dqx{tb}")
                    nc.vector.tensor_scalar_mul(dq, axm, 1.0 / 127.0)
                    dqx.append(dq)
                    nc.vector.tensor_scalar(
                        xs, xs, sxq, RND, op0=OP.mult, op1=OP.add
                    )
                    for hg in range(0, HI_N, 4):
                        pt4 = ptrans.tile([128, 4, 128], F32, tag="ptr")
                        for i in range(4):
                            hi = hg + i
                            nc.tensor.transpose(
                                pt4[:, i, :], xs[:, hi * 128 : (hi + 1) * 128], ident
                            )
                        nc.scalar.activation(
                            xqT[:, hg : hg + 4, tb * 128 : (tb + 1) * 128],
                            pt4, AF.Identity, bias=negrnd,
                        )

            sb_kv = w_scales_group(("k", "v"), psmall, ar1_out, "kv")
            w_quant(wab_kv, "k", 0, sb_kv, wtmp, wq8, wag_kv, 0)
            w_quant(wab_kv, "v", 1, sb_kv, wtmp, wq8, wag_kv, OK)
            nc.gpsimd.collective_compute(
                "AllGather", OP.bypass, replica_groups=[list(range(NC))],
                ins=[wag_kv.opt()], outs=[wint_kv.opt()],
            )
            rswb["k"] = sb_kv[:, 2:3]
            rswb["v"] = sb_kv[:, 3:4]
            sb_qo = w_scales_group(("q", "o"), psmall, ar2_out, "qo")
            w_quant(wab_qo, "q", 0, sb_qo, wtmp, wq8, wag_q, 0)
            nc.gpsimd.collective_compute(
                "AllGather", OP.bypass, replica_groups=[list(range(NC))],
                ins=[wag_q.opt()], outs=[wint_q.opt()],
            )
            w_quant(wab_qo, "o", 1, sb_qo, wtmp, wq8, wag_o, 0)
            rswb["q"] = sb_qo[:, 2:3]
            rswb["o"] = sb_qo[:, 3:4]

        # dequant vectors (absmax/127 * 1/s_w)
        dqv = {}
        for m in ("q", "k", "v", "o"):
            for tb in range(TB):
                d = vecs.tile([128, 1], F32, name=f"dqv_{m}{tb}")
                nc.vector.tensor_mul(d, dqx[tb], rswb[m])
                dqv[(m, tb)] = d

        q_sb = persist.tile([128, TB, OQ], F32, tag="qsb")
        k_sb = persist.tile([128, TB, OK], F32)
        v_loc = persist.tile([128, TB, NKV, 130], BF16)
        nc.vector.memset(v_loc, 1.0)
        qT = persist.tile([128, NH, T], BF16)  # [d, head, t]
        kT = persist.tile([128, NKV, T], BF16, tag="t8", padded_shape=[128, HI_N, T])

        def proj_load(wint_src, o_w, m, wpool):
            """Stream the fp8 weight matrix in 4 hi-chunks (pipelined)."""
            src3 = wint_src.rearrange("(hi p) o -> p hi o", p=128)
            chunks = []
            for cg in range(4):
                wst = wpool.tile(
                    [128, 4, o_w], FP8, tag="wst",
                    padded_shape=[128, 4, OQ], name=f"wst_{m}{cg}",
                )
                sync.dma_start(wst, src3[:, cg * 4 : (cg + 1) * 4, :])
                chunks.append(wst)
            return chunks

        def proj_tb(chunks, col0, o_w, m, tb, dst_fn, ppool):
            """dequant(xqT.T @ w_int) for one token tile over all o-chunks."""
            for oc in range(o_w // 512):
                pp = ppool.tile([128, 512], F32, tag="pp")
                for hi in range(HI_N):
                    nc.tensor.matmul(
                        pp,
                        xqT[:, hi, tb * 128 : (tb + 1) * 128],
                        chunks[hi // 4][
                            :, hi % 4, col0 + oc * 512 : col0 + (oc + 1) * 512
                        ],
                        start=(hi == 0),
                        stop=(hi == HI_N - 1),
                    )
                nc.vector.tensor_scalar(
                    dst_fn(tb, oc), pp, dqv[(m, tb)], None, op0=OP.mult
                )

        def rope_batch(src_sb, tb, nh, cosn, sinn, dstT, ropep, label):
            w = nh * 128
            blk = src_sb[:, tb, :]  # [128, w] f32
            sq = ropep.tile([128, w], F32, tag="unf", padded_shape=[128, NH * 128])
            nc.scalar.activation(sq, blk, AF.Square)
            ms = vecs.tile([128, nh], F32, name=f"ms_{label}{tb}")
            nc.vector.tensor_reduce(
                ms, sq.rearrange("p (h d) -> p h d", h=nh), axis=AX, op=OP.add
            )
            rms = vecs.tile([128, nh], F32, name=f"rms_{label}{tb}")
            nc.scalar.activation(rms, ms, AF.Sqrt, scale=1.0 / HD, bias=epsb)
            rn = vecs.tile([128, nh], F32, name=f"rn_{label}{tb}")
            nc.vector.reciprocal(rn, rms)
            rnb = rn.to_broadcast([128, nh, 128])
            blk3 = blk.rearrange("p (h d) -> p h d", h=nh)
            un2 = ropep.tile(
                [128, nh * 128], F32, tag="unf", padded_shape=[128, NH * 128],
                name="un2",
            )
            un = un2.rearrange("p (h d) -> p h d", h=nh)
            nc.vector.tensor_mul(un, blk3, rnb)
            cosb = (
                cs[cosn][:, tb, :]
                .rearrange("p (one d) -> p one d", one=1)
                .to_broadcast([128, nh, 128])
            )
            sinb = (
                cs[sinn][:, tb, :]
                .rearrange("p (one d) -> p one d", one=1)
                .to_broadcast([128, nh, 128])
            )
            ra = ropep.tile([128, nh, 128], F32, tag="ra", padded_shape=[128, NH, 128])
            nc.vector.tensor_mul(ra, un, cosb)
            rb = ropep.tile([128, nh, 128], F32, tag="rb", padded_shape=[128, NH, 128])
            nc.vector.tensor_mul(rb[:, :, 0:64], un[:, :, 64:128], sinb[:, :, 0:64])
            nc.vector.tensor_mul(rb[:, :, 64:128], un[:, :, 0:64], sinb[:, :, 64:128])
            nc.vector.tensor_add(ra, ra, rb)
            for hg in range(0, nh, 4):
                pt4 = ptrans.tile([128, 4, 128], F32, tag="ptr")
                for i in range(4):
                    nc.tensor.transpose(pt4[:, i, :], ra[:, hg + i, :], ident)
                nc.scalar.activation(
                    dstT[:, hg : hg + 4, tb * 128 : (tb + 1) * 128], pt4, AF.Copy
                )

        # ====== K/V projections + K rope + KV allgather ======
        with (
            tc.tile_pool(name="wmm1", bufs=8) as wmm1,
            tc.tile_pool(name="pproj1", bufs=3, space="PSUM") as pproj1,
            tc.tile_pool(name="ropek", bufs=1) as ropek,
        ):
            wst_k = proj_load(wint_kv[:, 0:OK], OK, "k", wmm1)
            for tb in range(TB):
                proj_tb(wst_k, 0, OK, "k", tb,
                        lambda tb, oc: k_sb[:, tb, oc * 512 : (oc + 1) * 512],
                        pproj1)
                rope_batch(k_sb, tb, NKV, "ck", "sk", kT, ropek, "k")
            sync.dma_start(
                k_in.rearrange("p (hk t) -> p hk t", hk=NKV), kT
            )
            nc.gpsimd.collective_compute(
                "AllGather", OP.bypass,
                replica_groups=[[0, 1, 2, 3], [4, 5, 6, 7]],
                ins=[k_in.opt()], outs=[k_out.opt()],
            )
            wst_v = proj_load(wint_kv[:, OK : OK + OV], OV, "v", wmm1)
            for tb in range(TB):
                proj_tb(wst_v, 0, OV, "v", tb,
                        lambda tb, oc: v_loc[:, tb, oc * 4 : (oc + 1) * 4, 0:128],
                        pproj1)
            sync.dma_start(
                v_in.rearrange("p (a hk d) -> p a hk d", a=TB, hk=NKV),
                v_loc[:, :, :, 0:128],
            )
            nc.gpsimd.collective_compute(
                "AllGather", OP.bypass,
                replica_groups=[[0, 1, 2, 3], [4, 5, 6, 7]],
                ins=[v_in.opt()], outs=[v_out.opt()],
            )

            # ====== Q projection + Q rope (overlap the KV allgathers) ======
            wst_q = proj_load(wint_q, OQ, "q", wmm1)
            for tb in range(TB):
                proj_tb(wst_q, 0, OQ, "q", tb,
                        lambda tb, oc: q_sb[:, tb, oc * 512 : (oc + 1) * 512],
                        pproj1)
                rope_batch(q_sb, tb, NH, "cq", "sq", qT, ropek, "q")
            nc.gpsimd.collective_compute(
                "AllGather", OP.bypass, replica_groups=[list(range(NC))],
                ins=[wag_o.opt()], outs=[wint_o.opt()],
            )

        mask_sb = persist.tile([128, KTILES + TB, T], BF16)
        sync.dma_start(mask_sb, mask_in.ap())
        attn = persist.tile([128, TB, H], F32, tag="qsb")  # reuse q_sb slot
        attn_loc = persist.tile([128, TB, NH, 132], F32)

        # prefetch o_proj weights under the attention phase
        wmm2 = tc.tile_pool(name="wmm2", bufs=1).__enter__()
        src3o = wint_o.rearrange("(hi p) o -> p hi o", p=128)
        wsto = wmm2.tile([128, HI_N, OO], FP8, tag="wst2")
        sync.dma_start(wsto, src3o)

        # ====== attention ======
        with (
            tc.tile_pool(name="pscore", bufs=2, space="PSUM") as pscore,
            tc.tile_pool(name="ppv", bufs=2, space="PSUM") as ppv,
            tc.tile_pool(name="pexp", bufs=4) as pexp,
        ):
            # local part: own K/V tiles (diagonal blocks) - no collective dep
            for h in range(NH):
                hk = h // 2
                pel = pexp.tile([128, TB, T], BF16, tag="pel")
                st = pscore.tile([128, 2, T], F32, tag="st")
                for a in range(TB):
                    nc.tensor.matmul(
                        st[:, a, :], kT[:, hk, a * 128 : (a + 1) * 128], qT[:, h, :],
                        start=True, stop=True,
                    )
                nc.scalar.activation(pel, st, AF.Exp, scale=INV_SQRT_HD)
                nc.vector.tensor_mul(pel, pel, mask_sb[:, KTILES : KTILES + TB, :])
                for tb in range(TB):
                    po = ppv.tile([128, 132], F32, tag="po", padded_shape=[128, 132])
                    for a in range(TB):
                        nc.tensor.matmul(
                            po[:, 0:129],
                            pel[:, a, tb * 128 : (tb + 1) * 128],
                            v_loc[:, a, hk, 0:129],
                            start=(a == 0),
                            stop=(a == TB - 1),
                        )
                    nc.vector.tensor_copy(attn_loc[:, tb, h, 0:129], po[:, 0:129])

            # gather readback
            kT_all = persist.tile([128, NKV, KTILES, 128], BF16)
            v_all = persist.tile([128, KTILES, NKV, 130], BF16)
            nc.vector.memset(v_all, 1.0)
            for cb in range(GROUP):
                # kT part: k_out row = 128*cb + d ; col = hk*256 + a*128 + t
                src_k = k_out[cb * 128 : (cb + 1) * 128, :].rearrange(
                    "d (hk t) -> d hk t", hk=NKV
                )
                sync.dma_start(kT_all[:, :, 2 * cb : 2 * cb + 2, :], src_k)
            for cb in range(GROUP):
                # v part: v_out row = 128*cb + p ; col = a*1024 + hk*128 + d
                src_v = v_out[cb * 128 : (cb + 1) * 128, :].rearrange(
                    "p (a hk d) -> p a hk d", a=TB, hk=NKV
                )
                sync.dma_start(v_all[:, 2 * cb : 2 * cb + 2, :, 0:128], src_v)

            # remote part: strictly-below-diagonal tiles from the allgather
            for h in range(NH):
                hk = h // 2
                pe = pexp.tile([128, KTILES, T], BF16, tag="pe")
                for jp in range(KTILES // 2):
                    st = pscore.tile([128, 2, T], F32, tag="st")
                    for i in range(2):
                        nc.tensor.matmul(
                            st[:, i, :], kT_all[:, hk, 2 * jp + i, :], qT[:, h, :],
                            start=True, stop=True,
                        )
                    nc.scalar.activation(
                        pe[:, 2 * jp : 2 * jp + 2, :], st, AF.Exp, scale=INV_SQRT_HD
                    )
                nc.vector.tensor_mul(pe, pe, mask_sb[:, 0:KTILES, :])
                for tb in range(TB):
                    po = ppv.tile([128, 132], F32, tag="po", padded_shape=[128, 132])
                    for j in range(KTILES):
                        nc.tensor.matmul(
                            po[:, 0:129],
                            pe[:, j, tb * 128 : (tb + 1) * 128],
                            v_all[:, j, hk, 0:129],
                            start=(j == 0),
                            stop=(j == KTILES - 1),
                        )
                    cmb = pexp.tile([128, 132], F32, tag="cmb")
                    nc.vector.tensor_add(
                        cmb[:, 0:129], po[:, 0:129], attn_loc[:, tb, h, 0:129]
                    )
                    rden = vecs.tile([128, 1], F32, name=f"rden{h}_{tb}")
                    nc.vector.reciprocal(rden, cmb[:, 128:129])
                    nc.vector.tensor_scalar(
                        attn[:, tb, h * 128 : (h + 1) * 128],
                        cmb[:, 0:128], rden, None, op0=OP.mult,
                    )

        # ====== act_quant(attn) + o_proj ======
        with (
            tc.tile_pool(name="oq", bufs=2) as oq,
            tc.tile_pool(name="pproj2", bufs=3, space="PSUM") as pproj2,
            tc.tile_pool(name="osb", bufs=2) as osb,
        ):
            aT = persist.tile([128, HI_N, T], BF16, tag="t8")
            dqo = []
            for tb in range(TB):
                axm = vecs.tile([128, 1], F32, name=f"oaxm{tb}")
                nc.vector.tensor_reduce(
                    axm, attn[:, tb, :], axis=AX, op=OP.max,
                    apply_absolute_value=True,
                )
                rsx = vecs.tile([128, 1], F32, name=f"orsx{tb}")
                nc.vector.reciprocal(rsx, axm)
                sxq = vecs.tile([128, 1], F32, name=f"osxq{tb}")
                nc.vector.tensor_scalar_mul(sxq, rsx, 127.0)
                dq = vecs.tile([128, 1], F32, name=f"odqx{tb}")
                nc.vector.tensor_scalar_mul(dq, axm, 1.0 / 127.0)
                d2 = vecs.tile([128, 1], F32, name=f"odq2{tb}")
                nc.vector.tensor_mul(d2, dq, rswb["o"])
                dqo.append(d2)
                ar = oq.tile([128, H], F32, tag="ar")
                nc.vector.tensor_scalar(
                    ar, attn[:, tb, :], sxq, RND, op0=OP.mult, op1=OP.add
                )
                for hg in range(0, HI_N, 4):
                    pt4 = ptrans.tile([128, 4, 128], F32, tag="ptr")
                    for i in range(4):
                        hi = hg + i
                        nc.tensor.transpose(
                            pt4[:, i, :], ar[:, hi * 128 : (hi + 1) * 128], ident
                        )
                    nc.scalar.activation(
                        aT[:, hg : hg + 4, tb * 128 : (tb + 1) * 128],
                        pt4, AF.Identity, bias=negrnd,
                    )

            for tb in range(TB):
                for oc in range(OO // 512):
                    pp = pproj2.tile([128, 512], F32, tag="pp2")
                    for hi in range(HI_N):
                        nc.tensor.matmul(
                            pp,
                            aT[:, hi, tb * 128 : (tb + 1) * 128],
                            wsto[:, hi, oc * 512 : (oc + 1) * 512],
                            start=(hi == 0),
                            stop=(hi == HI_N - 1),
                        )
                    ot = osb.tile([128, 512], F32, tag="ot")
                    nc.vector.tensor_scalar(ot, pp, dqo[tb], None, op0=OP.mult)
                    sync.dma_start(
                        out.ap()[
                            tb * 128 : (tb + 1) * 128, oc * 512 : (oc + 1) * 512
                        ],
                        ot,
                    )


def _host_inputs(x, cos, sin, wq, wk, wv, wo, qn, kn):
    """Build the 8 per-core input maps (pure slicing / layout transforms)."""
    x2 = np.asarray(x, np.float32).reshape(B * S, H)
    cos = np.asarray(cos, np.float32)
    sin = np.asarray(sin, np.float32)
    qn = np.asarray(qn, np.float32)
    kn = np.asarray(kn, np.float32)
    # fold qk-norm weights into rope tables (exact identity when qn=kn=1)
    qn_rot = np.concatenate([qn[HD // 2 :], qn[: HD // 2]])
    kn_rot = np.concatenate([kn[HD // 2 :], kn[: HD // 2]])
    sgn = np.concatenate(
        [-np.ones(HD // 2, np.float32), np.ones(HD // 2, np.float32)]
    )
    cosq_t = cos * qn[None, :]
    sinq_t = sin * (qn_rot * sgn)[None, :]
    cosk_t = cos * kn[None, :]
    sink_t = sin * (kn_rot * sgn)[None, :]

    wt = {
        "q": np.asarray(wq, np.float32).T,  # [H, OQ]
        "k": np.asarray(wk, np.float32).T,
        "v": np.asarray(wv, np.float32).T,
        "o": np.asarray(wo, np.float32).T,  # [H(=in), OO]
    }
    worder = ("k", "v", "q", "o")
    wconst = np.concatenate(
        [
            np.array([WNUMEL[m] for m in worder], np.float32),
            np.array([1.0 / WNUMEL[m] for m in worder], np.float32),
        ]
    ).reshape(1, 8)

    in_maps = []
    for c in range(NC):
        qt = c % GROUP
        t0 = qt * T
        # strict mask [p, j, f]: key (128j+p) fully below this core's window
        p = np.arange(128)[:, None, None]
        j = np.arange(KTILES)[None, :, None]
        f = np.arange(T)[None, None, :]
        strict = ((128 * j + p) < t0) & (f >= 0)
        # diagonal masks for the two local key tiles
        a = np.arange(TB)[None, :, None]
        diag = (128 * a + p) <= f
        mask = np.concatenate([strict, diag], axis=1).astype(ml_dtypes.bfloat16)
        m = {
            "x_sl": np.ascontiguousarray(x2[c * T : (c + 1) * T]),
            "cosq": np.ascontiguousarray(cosq_t[t0 : t0 + T]),
            "sinq": np.ascontiguousarray(sinq_t[t0 : t0 + T]),
            "cosk": np.ascontiguousarray(cosk_t[t0 : t0 + T]),
            "sink": np.ascontiguousarray(sink_t[t0 : t0 + T]),
            "wq_sl": np.ascontiguousarray(wt["q"][c * HSL : (c + 1) * HSL]),
            "wk_sl": np.ascontiguousarray(wt["k"][c * HSL : (c + 1) * HSL]),
            "wv_sl": np.ascontiguousarray(wt["v"][c * HSL : (c + 1) * HSL]),
            "wo_sl": np.ascontiguousarray(wt["o"][c * HSL : (c + 1) * HSL]),
            "mask": mask,
            "wconst": wconst,
        }
        in_maps.append(m)
    return in_maps


def kernel(x, cos, sin, wq, wk, wv, wo, qn, kn):
    if "nc" not in _CACHE:
        _CACHE["nc"] = _build()
    nc = _CACHE["nc"]
    in_maps = _host_inputs(x, cos, sin, wq, wk, wv, wo, qn, kn)
    res = bass_utils.run_bass_kernel_spmd(nc, in_maps, core_ids=list(range(NC)))
    outs = [np.asarray(res.results[c]["out"]) for c in range(NC)]
    return np.concatenate(outs, axis=0).reshape(B, S, H).astype(np.float32)


# revision 45
# speedup vs baseline: 1.0339x; 1.0256x over previous
"""BitNet GQA attention layer on 8 TRN2 NeuronCores.

Sharding: token-parallel. B*S = 2048 tokens -> 256 per core (core c: batch
c//4, quarter c%4). Weights are split 8-way along the contraction dim for
quantization (exact global absmean via tiny AllReduces), then the ternary
integer weights are AllGathered in bf16 (three pipelined AGs: k+v first so
K/V projections start early, then q, then o). K/V are AllGathered within
each batch's 4-core group, hidden under Q-projection + Q-rope. All BitNet
matmuls run as exact integer arithmetic in bf16 (acts in [-128,127],
weights in {-1,0,1}) with fp32 PSUM accumulation; dequantization scales are
applied to the fp32 results.
"""

import sys

sys.path.insert(0, "/opt/trn_rl_repo")

import numpy as np
import ml_dtypes

import concourse.bass as bass
import concourse.mybir as mybir
import concourse.tile as tile
from concourse import bacc
from concourse import bass_utils
from concourse.masks import make_identity

F32 = mybir.dt.float32
BF16 = mybir.dt.bfloat16
FP8 = mybir.dt.float8e4
AX = mybir.AxisListType.X
OP = mybir.AluOpType
AF = mybir.ActivationFunctionType

B, S, H = 2, 1024, 2048
NH, NKV, HD = 16, 8, 128
NC = 8
T = (B * S) // NC  # 256 tokens per core
TB = T // 128  # 2 token tiles per core
HSL = H // NC  # 256 weight rows per core
EPS = 1e-6
RND = 12582912.0  # 1.5 * 2**23: fp32 add => round-to-nearest-even
INV_SQRT_HD = 1.0 / float(np.sqrt(HD))
KTILES = S // 128  # 8 key tiles per batch
GROUP = 4  # cores per batch

OQ, OK, OV, OO = H, NKV * HD, NKV * HD, H  # 2048, 1024, 1024, 2048
OW = {"q": OQ, "k": OK, "v": OV, "o": OO}
WNUMEL = {m: OW[m] * H for m in OW}
HI_N = H // 128  # 16 contraction tiles

_CACHE = {}


def _build():
    nc = bacc.Bacc("TRN2", target_bir_lowering=False, debug=False, num_devices=NC)

    x_sl = nc.dram_tensor("x_sl", [T, H], F32, kind="ExternalInput")
    cosq = nc.dram_tensor("cosq", [T, HD], F32, kind="ExternalInput")
    sinq = nc.dram_tensor("sinq", [T, HD], F32, kind="ExternalInput")
    cosk = nc.dram_tensor("cosk", [T, HD], F32, kind="ExternalInput")
    sink = nc.dram_tensor("sink", [T, HD], F32, kind="ExternalInput")
    w_sl = {
        "q": nc.dram_tensor("wq_sl", [HSL, OQ], F32, kind="ExternalInput"),
        "k": nc.dram_tensor("wk_sl", [HSL, OK], F32, kind="ExternalInput"),
        "v": nc.dram_tensor("wv_sl", [HSL, OV], F32, kind="ExternalInput"),
        "o": nc.dram_tensor("wo_sl", [HSL, OO], F32, kind="ExternalInput"),
    }
    mask_in = nc.dram_tensor("mask", [128, KTILES + TB, T], BF16, kind="ExternalInput")
    # cols 0-3: numel for k,v,q,o ; cols 4-7: 1/numel for k,v,q,o
    wconst = nc.dram_tensor("wconst", [1, 8], F32, kind="ExternalInput")
    out = nc.dram_tensor("out", [T, H], F32, kind="ExternalOutput")

    with tile.TileContext(nc) as tc:
        _build_body(nc, tc, x_sl, cosq, sinq, cosk, sink, w_sl, mask_in, wconst, out)

    nc.compile()
    return nc


def _build_body(nc, tc, x_sl, cosq, sinq, cosk, sink, w_sl, mask_in, wconst, out):
    sync = nc.sync

    with (
        tc.tile_pool(name="dram", bufs=1, space="DRAM") as dram,
        tc.tile_pool(name="const", bufs=1) as constp,
        tc.tile_pool(name="vecs", bufs=1) as vecs,
        tc.tile_pool(name="persist", bufs=1) as persist,
        tc.tile_pool(name="ptrans", bufs=2, space="PSUM") as ptrans,
    ):
        # ---- DRAM bounce buffers for collectives ----
        wag_k = dram.tile([HSL, OK], FP8)
        wint_k = dram.tile([H, OK], FP8, addr_space="Shared")
        wag_v = dram.tile([HSL, OV], FP8)
        wint_v = dram.tile([H, OV], FP8, addr_space="Shared")
        wag_q = dram.tile([HSL, OQ], FP8)
        wint_q = dram.tile([H, OQ], FP8, addr_space="Shared")
        wag_o = dram.tile([HSL, OO], FP8)
        wint_o = dram.tile([H, OO], FP8, addr_space="Shared")
        ar1_in = dram.tile([1, 8], F32)
        ar1_out = dram.tile([1, 8], F32, addr_space="Shared")
        ar2_in = dram.tile([1, 8], F32)
        ar2_out = dram.tile([1, 8], F32, addr_space="Shared")
        k_in = dram.tile([128, NKV * T], BF16)
        k_out = dram.tile([512, NKV * T], BF16)
        v_in = dram.tile([128, TB * OV], BF16)
        v_out = dram.tile([512, TB * OV], BF16)

        # warmup: absorb the collective engine's cold-start latency
        warm_in = dram.tile([1, 8], F32)
        warm_out = dram.tile([1, 8], F32, addr_space="Shared")
        wz = constp.tile([1, 8], F32)
        nc.vector.memset(wz, 0.0)
        nc.scalar.dma_start(warm_in, wz)
        nc.gpsimd.collective_compute(
            "AllReduce", OP.add, replica_groups=[list(range(NC))],
            ins=[warm_in.opt()], outs=[warm_out.opt()],
        )

        # ---- constants ----
        ident = constp.tile([128, 128], F32)
        make_identity(nc, ident)
        ones1 = constp.tile([1, 128], F32)
        nc.vector.memset(ones1, 1.0)
        onescol = constp.tile([128, 1], F32)
        nc.vector.memset(onescol, 1.0)
        wconst_sb = constp.tile([1, 8], F32)
        sync.dma_start(wconst_sb, wconst.ap())
        negrnd = constp.tile([128, 1], F32)
        nc.vector.memset(negrnd, -RND)
        epsb = constp.tile([128, 1], F32)
        nc.vector.memset(epsb, EPS)
        cs = {}
        for nm, t in (("cq", cosq), ("sq", sinq), ("ck", cosk), ("sk", sink)):
            c = constp.tile([128, TB, HD], F32, name=f"cs_{nm}")
            sync.dma_start(c, t.ap().rearrange("(a p) d -> p a d", p=128))
            cs[nm] = c
        # persistent activations
        xqT = persist.tile([128, HI_N, T], BF16)  # [h%128, h//128, t]

        # ====== Phase W: weight scales + quantize + pipelined allgathers ======
        def w_sums_group(mats, psmall, ar_in, ar_out, wraws, label):
            """Load slices of `mats`, abs-sum, kick the AllReduce."""
            wab = {}
            for m in mats:
                for pt in range(2):
                    wr = wraws[m].tile([128, OW[m]], F32, name=f"wr_{m}{pt}")
                    sync.dma_start(wr, w_sl[m].ap()[pt * 128 : (pt + 1) * 128, :])
                    wab[(m, pt)] = wr
            red0 = vecs.tile([128, 2], F32, name=f"red0_{label}")
            red1 = vecs.tile([128, 2], F32, name=f"red1_{label}")
            for mi, m in enumerate(mats):
                for pt, red in ((0, red0), (1, red1)):
                    nc.vector.tensor_reduce(
                        red[:, mi : mi + 1], wab[(m, pt)], axis=AX, op=OP.add,
                        apply_absolute_value=True,
                    )
            redc = vecs.tile([128, 2], F32, name=f"redc_{label}")
            nc.vector.tensor_add(redc, red0, red1)
            ps = psmall.tile([1, 2], F32, name=f"ps_{label}", tag="psm")
            nc.tensor.matmul(ps, onescol, redc, start=True, stop=True)
            sums = vecs.tile([1, 8], F32, name=f"sums_{label}")
            nc.vector.memset(sums, 0.0)
            nc.scalar.copy(sums[:, 0:2], ps)
            nc.scalar.dma_start(ar_in, sums)
            nc.gpsimd.collective_compute(
                "AllReduce", OP.add, replica_groups=[list(range(NC))],
                ins=[ar_in.opt()], outs=[ar_out.opt()],
            )
            return wab

        def w_scales_group(mats, psmall, ar_out, label):
            """Read back the AllReduce and build the [128,4] scale tile."""
            g = vecs.tile([1, 8], F32, name=f"g_{label}")
            nc.scalar.dma_start(g, ar_out)
            r2 = vecs.tile([1, 2], F32, name=f"r2_{label}")
            nc.vector.reciprocal(r2, g[:, 0:2])
            sw4 = vecs.tile([1, 4], F32, name=f"sw4_{label}")
            ncol = {("k", "v"): (0, 2), ("q", "o"): (2, 4)}[tuple(mats)]
            nc.vector.tensor_mul(sw4[:, 0:2], r2, wconst_sb[:, ncol[0] : ncol[1]])
            nc.vector.tensor_mul(
                sw4[:, 2:4], g[:, 0:2], wconst_sb[:, 4 + ncol[0] : 4 + ncol[1]]
            )
            pb = psmall.tile([128, 4], F32, name=f"pb_{label}", tag="psm")
            nc.tensor.matmul(pb, ones1, sw4, start=True, stop=True)
            sb = vecs.tile([128, 4], F32, name=f"sb_{label}")
            nc.scalar.copy(sb, pb)
            return sb

        def w_quant(wab, m, mi, sb, wtmp, wq8, dst, col0):
            for pt in range(2):
                wr = wab[(m, pt)]
                tmp = wtmp.tile([128, OW[m]], F32, tag="wtmp")
                nc.vector.tensor_scalar(
                    tmp, wr, sb[:, mi : mi + 1], RND, op0=OP.mult, op1=OP.add
                )
                nc.vector.tensor_scalar(
                    tmp, tmp, -RND, 1.0, op0=OP.add, op1=OP.min
                )
                wi = wq8.tile([128, OW[m]], FP8, tag="wi")
                nc.vector.tensor_scalar(wi, tmp, -1.0, None, op0=OP.max)
                nc.scalar.dma_start(
                    dst[pt * 128 : (pt + 1) * 128, col0 : col0 + OW[m]], wi
                )

        rswb = {}
        with (
            tc.tile_pool(name="wraw_q", bufs=1) as wraw_q,
            tc.tile_pool(name="wraw_k", bufs=1) as wraw_k,
            tc.tile_pool(name="wraw_v", bufs=1) as wraw_v,
            tc.tile_pool(name="wraw_o", bufs=1) as wraw_o,
            tc.tile_pool(name="wtmp", bufs=2) as wtmp,
            tc.tile_pool(name="wq8", bufs=2) as wq8,
            tc.tile_pool(name="psmall", bufs=2, space="PSUM") as psmall,
        ):
            wraws = {"q": wraw_q, "k": wraw_k, "v": wraw_v, "o": wraw_o}
            wab_kv = w_sums_group(("k", "v"), psmall, ar1_in, ar1_out, wraws, "kv")
            # x loads right behind wk/wv so the x pipeline fills the AR window
            xs_t = []
            with tc.tile_pool(name="xraw", bufs=2) as xraw:
                for tb in range(TB):
                    xs = xraw.tile([128, H], F32, tag="xs", name=f"xs{tb}")
                    sync.dma_start(xs, x_sl.ap()[tb * 128 : (tb + 1) * 128, :])
                    xs_t.append(xs)
                wab_qo = w_sums_group(
                    ("q", "o"), psmall, ar2_in, ar2_out, wraws, "qo"
                )

                # ====== Phase X: act_quant(x) + transpose ======
                dqx = []
                for tb in range(TB):
                    xs = xs_t[tb]
                    axm = vecs.tile([128, 1], F32, name=f"axm{tb}")
                    nc.vector.tensor_reduce(
                        axm, xs, axis=AX, op=OP.max, apply_absolute_value=True
                    )
                    rsx = vecs.tile([128, 1], F32, name=f"rsx{tb}")
                    nc.vector.reciprocal(rsx, axm)
                    sxq = vecs.tile([128, 1], F32, name=f"sxq{tb}")
                    nc.vector.tensor_scalar_mul(sxq, rsx, 127.0)
                    dq = vecs.tile([128, 1], F32, name=f"dqx{tb}")
                    nc.vector.tensor_scalar_mul(dq, axm, 1.0 / 127.0)
                    dqx.append(dq)
                    nc.vector.tensor_scalar(
                        xs, xs, sxq, RND, op0=OP.mult, op1=OP.add
                    )
                    for hg in range(0, HI_N, 4):
                        pt4 = ptrans.tile([128, 4, 128], F32, tag="ptr")
                        for i in range(4):
                            hi = hg + i
                            nc.tensor.transpose(
                                pt4[:, i, :], xs[:, hi * 128 : (hi + 1) * 128], ident
                            )
                        nc.scalar.activation(
                            xqT[:, hg : hg + 4, tb * 128 : (tb + 1) * 128],
                            pt4, AF.Identity, bias=negrnd,
                        )

            sb_kv = w_scales_group(("k", "v"), psmall, ar1_out, "kv")
            w_quant(wab_kv, "k", 0, sb_kv, wtmp, wq8, wag_k, 0)
            nc.gpsimd.collective_compute(
                "AllGather", OP.bypass, replica_groups=[list(range(NC))],
                ins=[wag_k.opt()], outs=[wint_k.opt()],
            )
            w_quant(wab_kv, "v", 1, sb_kv, wtmp, wq8, wag_v, 0)
            nc.gpsimd.collective_compute(
                "AllGather", OP.bypass, replica_groups=[list(range(NC))],
                ins=[wag_v.opt()], outs=[wint_v.opt()],
            )
            rswb["k"] = sb_kv[:, 2:3]
            rswb["v"] = sb_kv[:, 3:4]
            sb_qo = w_scales_group(("q", "o"), psmall, ar2_out, "qo")
            w_quant(wab_qo, "q", 0, sb_qo, wtmp, wq8, wag_q, 0)
            nc.gpsimd.collective_compute(
                "AllGather", OP.bypass, replica_groups=[list(range(NC))],
                ins=[wag_q.opt()], outs=[wint_q.opt()],
            )
            w_quant(wab_qo, "o", 1, sb_qo, wtmp, wq8, wag_o, 0)
            rswb["q"] = sb_qo[:, 2:3]
            rswb["o"] = sb_qo[:, 3:4]

        # dequant vectors (absmax/127 * 1/s_w)
        dqv = {}
        for m in ("q", "k", "v", "o"):
            for tb in range(TB):
                d = vecs.tile([128, 1], F32, name=f"dqv_{m}{tb}")
                nc.vector.tensor_mul(d, dqx[tb], rswb[m])
                dqv[(m, tb)] = d

        q_sb = persist.tile([128, TB, OQ], F32, tag="qsb")
        k_sb = persist.tile([128, TB, OK], F32)
        v_loc = persist.tile([128, TB, NKV, 130], BF16)
        nc.vector.memset(v_loc, 1.0)
        qT = persist.tile([128, NH, T], BF16)  # [d, head, t]
        kT = persist.tile([128, NKV, T], BF16, tag="t8", padded_shape=[128, HI_N, T])

        def proj_load(wint_src, o_w, m, wpool):
            """Stream the fp8 weight matrix in 4 hi-chunks (pipelined)."""
            src3 = wint_src.rearrange("(hi p) o -> p hi o", p=128)
            chunks = []
            for cg in range(4):
                wst = wpool.tile(
                    [128, 4, o_w], FP8, tag="wst",
                    padded_shape=[128, 4, OQ], name=f"wst_{m}{cg}",
                )
                sync.dma_start(wst, src3[:, cg * 4 : (cg + 1) * 4, :])
                chunks.append(wst)
            return chunks

        def proj_tb(chunks, col0, o_w, m, tb, dst_fn, ppool):
            """dequant(xqT.T @ w_int) for one token tile over all o-chunks."""
            for oc in range(o_w // 512):
                pp = ppool.tile([128, 512], F32, tag="pp")
                for hi in range(HI_N):
                    nc.tensor.matmul(
                        pp,
                        xqT[:, hi, tb * 128 : (tb + 1) * 128],
                        chunks[hi // 4][
                            :, hi % 4, col0 + oc * 512 : col0 + (oc + 1) * 512
                        ],
                        start=(hi == 0),
                        stop=(hi == HI_N - 1),
                    )
                nc.vector.tensor_scalar(
                    dst_fn(tb, oc), pp, dqv[(m, tb)], None, op0=OP.mult
                )

        def rope_batch(src_sb, tb, nh, cosn, sinn, dstT, ropep, label):
            w = nh * 128
            blk = src_sb[:, tb, :]  # [128, w] f32
            sq = ropep.tile([128, w], F32, tag="unf", padded_shape=[128, NH * 128])
            nc.scalar.activation(sq, blk, AF.Square)
            ms = vecs.tile([128, nh], F32, name=f"ms_{label}{tb}")
            nc.vector.tensor_reduce(
                ms, sq.rearrange("p (h d) -> p h d", h=nh), axis=AX, op=OP.add
            )
            rms = vecs.tile([128, nh], F32, name=f"rms_{label}{tb}")
            nc.scalar.activation(rms, ms, AF.Sqrt, scale=1.0 / HD, bias=epsb)
            rn = vecs.tile([128, nh], F32, name=f"rn_{label}{tb}")
            nc.vector.reciprocal(rn, rms)
            rnb = rn.to_broadcast([128, nh, 128])
            blk3 = blk.rearrange("p (h d) -> p h d", h=nh)
            un2 = ropep.tile(
                [128, nh * 128], F32, tag="unf", padded_shape=[128, NH * 128],
                name="un2",
            )
            un = un2.rearrange("p (h d) -> p h d", h=nh)
            nc.vector.tensor_mul(un, blk3, rnb)
            cosb = (
                cs[cosn][:, tb, :]
                .rearrange("p (one d) -> p one d", one=1)
                .to_broadcast([128, nh, 128])
            )
            sinb = (
                cs[sinn][:, tb, :]
                .rearrange("p (one d) -> p one d", one=1)
                .to_broadcast([128, nh, 128])
            )
            ra = ropep.tile([128, nh, 128], F32, tag="ra", padded_shape=[128, NH, 128])
            nc.vector.tensor_mul(ra, un, cosb)
            rb = ropep.tile([128, nh, 128], F32, tag="rb", padded_shape=[128, NH, 128])
            nc.vector.tensor_mul(rb[:, :, 0:64], un[:, :, 64:128], sinb[:, :, 0:64])
            nc.vector.tensor_mul(rb[:, :, 64:128], un[:, :, 0:64], sinb[:, :, 64:128])
            nc.vector.tensor_add(ra, ra, rb)
            for hg in range(0, nh, 4):
                pt4 = ptrans.tile([128, 4, 128], F32, tag="ptr")
                for i in range(4):
                    nc.tensor.transpose(pt4[:, i, :], ra[:, hg + i, :], ident)
                nc.scalar.activation(
                    dstT[:, hg : hg + 4, tb * 128 : (tb + 1) * 128], pt4, AF.Copy
                )

        # ====== K/V projections + K rope + KV allgather ======
        with (
            tc.tile_pool(name="wmm1", bufs=8) as wmm1,
            tc.tile_pool(name="pproj1", bufs=3, space="PSUM") as pproj1,
            tc.tile_pool(name="ropek", bufs=1) as ropek,
        ):
            wst_k = proj_load(wint_k, OK, "k", wmm1)
            for tb in range(TB):
                proj_tb(wst_k, 0, OK, "k", tb,
                        lambda tb, oc: k_sb[:, tb, oc * 512 : (oc + 1) * 512],
                        pproj1)
                rope_batch(k_sb, tb, NKV, "ck", "sk", kT, ropek, "k")
            sync.dma_start(
                k_in.rearrange("p (hk t) -> p hk t", hk=NKV), kT
            )
            nc.gpsimd.collective_compute(
                "AllGather", OP.bypass,
                replica_groups=[[0, 1, 2, 3], [4, 5, 6, 7]],
                ins=[k_in.opt()], outs=[k_out.opt()],
            )
            wst_v = proj_load(wint_v, OV, "v", wmm1)
            for tb in range(TB):
                proj_tb(wst_v, 0, OV, "v", tb,
                        lambda tb, oc: v_loc[:, tb, oc * 4 : (oc + 1) * 4, 0:128],
                        pproj1)
            sync.dma_start(
                v_in.rearrange("p (a hk d) -> p a hk d", a=TB, hk=NKV),
                v_loc[:, :, :, 0:128],
            )
            nc.gpsimd.collective_compute(
                "AllGather", OP.bypass,
                replica_groups=[[0, 1, 2, 3], [4, 5, 6, 7]],
                ins=[v_in.opt()], outs=[v_out.opt()],
            )

            # ====== Q projection + Q rope (overlap the KV allgathers) ======
            wst_q = proj_load(wint_q, OQ, "q", wmm1)
            for tb in range(TB):
                proj_tb(wst_q, 0, OQ, "q", tb,
                        lambda tb, oc: q_sb[:, tb, oc * 512 : (oc + 1) * 512],
                        pproj1)
                rope_batch(q_sb, tb, NH, "cq", "sq", qT, ropek, "q")
            nc.gpsimd.collective_compute(
                "AllGather", OP.bypass, replica_groups=[list(range(NC))],
                ins=[wag_o.opt()], outs=[wint_o.opt()],
            )

        mask_sb = persist.tile([128, KTILES + TB, T], BF16)
        sync.dma_start(mask_sb, mask_in.ap())
        attn = persist.tile([128, TB, H], F32, tag="qsb")  # reuse q_sb slot
        attn_loc = persist.tile([128, TB, NH, 132], F32)

        # prefetch o_proj weights under the attention phase
        wmm2 = tc.tile_pool(name="wmm2", bufs=1).__enter__()
        src3o = wint_o.rearrange("(hi p) o -> p hi o", p=128)
        wsto = wmm2.tile([128, HI_N, OO], FP8, tag="wst2")
        sync.dma_start(wsto, src3o)

        # ====== attention ======
        with (
            tc.tile_pool(name="pscore", bufs=2, space="PSUM") as pscore,
            tc.tile_pool(name="ppv", bufs=2, space="PSUM") as ppv,
            tc.tile_pool(name="pexp", bufs=4) as pexp,
        ):
            # local part: own K/V tiles (diagonal blocks) - no collective dep
            for h in range(NH):
                hk = h // 2
                pel = pexp.tile([128, TB, T], BF16, tag="pel")
                st = pscore.tile([128, 2, T], F32, tag="st")
                for a in range(TB):
                    nc.tensor.matmul(
                        st[:, a, :], kT[:, hk, a * 128 : (a + 1) * 128], qT[:, h, :],
                        start=True, stop=True,
                    )
                nc.scalar.activation(pel, st, AF.Exp, scale=INV_SQRT_HD)
                nc.vector.tensor_mul(pel, pel, mask_sb[:, KTILES : KTILES + TB, :])
                for tb in range(TB):
                    po = ppv.tile([128, 132], F32, tag="po", padded_shape=[128, 132])
                    for a in range(TB):
                        nc.tensor.matmul(
                            po[:, 0:129],
                            pel[:, a, tb * 128 : (tb + 1) * 128],
                            v_loc[:, a, hk, 0:129],
                            start=(a == 0),
                            stop=(a == TB - 1),
                        )
                    nc.vector.tensor_copy(attn_loc[:, tb, h, 0:129], po[:, 0:129])

            # gather readback
            kT_all = persist.tile([128, NKV, KTILES, 128], BF16)
            v_all = persist.tile([128, KTILES, NKV, 130], BF16)
            nc.vector.memset(v_all, 1.0)
            for cb in range(GROUP):
                # kT part: k_out row = 128*cb + d ; col = hk*256 + a*128 + t
                src_k = k_out[cb * 128 : (cb + 1) * 128, :].rearrange(
                    "d (hk t) -> d hk t", hk=NKV
                )
                sync.dma_start(kT_all[:, :, 2 * cb : 2 * cb + 2, :], src_k)
            for cb in range(GROUP):
                # v part: v_out row = 128*cb + p ; col = a*1024 + hk*128 + d
                src_v = v_out[cb * 128 : (cb + 1) * 128, :].rearrange(
                    "p (a hk d) -> p a hk d", a=TB, hk=NKV
                )
                sync.dma_start(v_all[:, 2 * cb : 2 * cb + 2, :, 0:128], src_v)

            # remote part: strictly-below-diagonal tiles from the allgather
            for h in range(NH):
                hk = h // 2
                pe = pexp.tile([128, KTILES, T], BF16, tag="pe")
                for jp in range(KTILES // 2):
                    st = pscore.tile([128, 2, T], F32, tag="st")
                    for i in range(2):
                        nc.tensor.matmul(
                            st[:, i, :], kT_all[:, hk, 2 * jp + i, :], qT[:, h, :],
                            start=True, stop=True,
                        )
                    nc.scalar.activation(
                        pe[:, 2 * jp : 2 * jp + 2, :], st, AF.Exp, scale=INV_SQRT_HD
                    )
                nc.vector.tensor_mul(pe, pe, mask_sb[:, 0:KTILES, :])
                for tb in range(TB):
                    po = ppv.tile([128, 132], F32, tag="po", padded_shape=[128, 132])
                    for j in range(KTILES):
                        nc.tensor.matmul(
                            po[:, 0:129],
                            pe[:, j, tb * 128 : (tb + 1) * 128],
                            v_all[:, j, hk, 0:129],
                            start=(j == 0),
                            stop=(j == KTILES - 1),
                        )
                    cmb = pexp.tile([128, 132], F32, tag="cmb")
                    nc.vector.tensor_add(
                        cmb[:, 0:129], po[:, 0:129], attn_loc[:, tb, h, 0:129]
                    )
                    rden = vecs.tile([128, 1], F32, name=f"rden{h}_{tb}")
                    nc.vector.reciprocal(rden, cmb[:, 128:129])
                    nc.vector.tensor_scalar(
                        attn[:, tb, h * 128 : (h + 1) * 128],
                        cmb[:, 0:128], rden, None, op0=OP.mult,
                    )

        # ====== act_quant(attn) + o_proj ======
        with (
            tc.tile_pool(name="oq", bufs=2) as oq,
            tc.tile_pool(name="pproj2", bufs=3, space="PSUM") as pproj2,
            tc.tile_pool(name="osb", bufs=2) as osb,
        ):
            aT = persist.tile([128, HI_N, T], BF16, tag="t8")
            dqo = []
            for tb in range(TB):
                axm = vecs.tile([128, 1], F32, name=f"oaxm{tb}")
                nc.vector.tensor_reduce(
                    axm, attn[:, tb, :], axis=AX, op=OP.max,
                    apply_absolute_value=True,
                )
                rsx = vecs.tile([128, 1], F32, name=f"orsx{tb}")
                nc.vector.reciprocal(rsx, axm)
                sxq = vecs.tile([128, 1], F32, name=f"osxq{tb}")
                nc.vector.tensor_scalar_mul(sxq, rsx, 127.0)
                dq = vecs.tile([128, 1], F32, name=f"odqx{tb}")
                nc.vector.tensor_scalar_mul(dq, axm, 1.0 / 127.0)
                d2 = vecs.tile([128, 1], F32, name=f"odq2{tb}")
                nc.vector.tensor_mul(d2, dq, rswb["o"])
                dqo.append(d2)
                ar = oq.tile([128, H], F32, tag="ar")
                nc.vector.tensor_scalar(
                    ar, attn[:, tb, :], sxq, RND, op0=OP.mult, op1=OP.add
                )
                for hg in range(0, HI_N, 4):
                    pt4 = ptrans.tile([128, 4, 128], F32, tag="ptr")
                    for i in range(4):
                        hi = hg + i
                        nc.tensor.transpose(
                            pt4[:, i, :], ar[:, hi * 128 : (hi + 1) * 128], ident
                        )
                    nc.scalar.activation(
                        aT[:, hg : hg + 4, tb * 128 : (tb + 1) * 128],
                        pt4, AF.Identity, bias=negrnd,
                    )

            for tb in range(TB):
                for oc in range(OO // 512):
                    pp = pproj2.tile([128, 512], F32, tag="pp2")
                    for hi in range(HI_N):
                        nc.tensor.matmul(
                            pp,
                            aT[:, hi, tb * 128 : (tb + 1) * 128],
                            wsto[:, hi, oc * 512 : (oc + 1) * 512],
                            start=(hi == 0),
                            stop=(hi == HI_N - 1),
                        )
                    ot = osb.tile([128, 512], F32, tag="ot")
                    nc.vector.tensor_scalar(ot, pp, dqo[tb], None, op0=OP.mult)
                    sync.dma_start(
                        out.ap()[
                            tb * 128 : (tb + 1) * 128, oc * 512 : (oc + 1) * 512
                        ],
                        ot,
                    )


def _host_inputs(x, cos, sin, wq, wk, wv, wo, qn, kn):
    """Build the 8 per-core input maps (pure slicing / layout transforms)."""
    x2 = np.asarray(x, np.float32).reshape(B * S, H)
    cos = np.asarray(cos, np.float32)
    sin = np.asarray(sin, np.float32)
    qn = np.asarray(qn, np.float32)
    kn = np.asarray(kn, np.float32)
    # fold qk-norm weights into rope tables (exact identity when qn=kn=1)
    qn_rot = np.concatenate([qn[HD // 2 :], qn[: HD // 2]])
    kn_rot = np.concatenate([kn[HD // 2 :], kn[: HD // 2]])
    sgn = np.concatenate(
        [-np.ones(HD // 2, np.float32), np.ones(HD // 2, np.float32)]
    )
    cosq_t = cos * qn[None, :]
    sinq_t = sin * (qn_rot * sgn)[None, :]
    cosk_t = cos * kn[None, :]
    sink_t = sin * (kn_rot * sgn)[None, :]

    wt = {
        "q": np.asarray(wq, np.float32).T,  # [H, OQ]
        "k": np.asarray(wk, np.float32).T,
        "v": np.asarray(wv, np.float32).T,
        "o": np.asarray(wo, np.float32).T,  # [H(=in), OO]
    }
    worder = ("k", "v", "q", "o")
    wconst = np.concatenate(
        [
            np.array([WNUMEL[m] for m in worder], np.float32),
            np.array([1.0 / WNUMEL[m] for m in worder], np.float32),
        ]
    ).reshape(1, 8)

    in_maps = []
    for c in range(NC):
        qt = c % GROUP
        t0 = qt * T
        # strict mask [p, j, f]: key (128j+p) fully below this core's window
        p = np.arange(128)[:, None, None]
        j = np.arange(KTILES)[None, :, None]
        f = np.arange(T)[None, None, :]
        strict = ((128 * j + p) < t0) & (f >= 0)
        # diagonal masks for the two local key tiles
        a = np.arange(TB)[None, :, None]
        diag = (128 * a + p) <= f
        mask = np.concatenate([strict, diag], axis=1).astype(ml_dtypes.bfloat16)
        m = {
            "x_sl": np.ascontiguousarray(x2[c * T : (c + 1) * T]),
            "cosq": np.ascontiguousarray(cosq_t[t0 : t0 + T]),
            "sinq": np.ascontiguousarray(sinq_t[t0 : t0 + T]),
            "cosk": np.ascontiguousarray(cosk_t[t0 : t0 + T]),
            "sink": np.ascontiguousarray(sink_t[t0 : t0 + T]),
            "wq_sl": np.ascontiguousarray(wt["q"][c * HSL : (c + 1) * HSL]),
            "wk_sl": np.ascontiguousarray(wt["k"][c * HSL : (c + 1) * HSL]),
            "wv_sl": np.ascontiguousarray(wt["v"][c * HSL : (c + 1) * HSL]),
            "wo_sl": np.ascontiguousarray(wt["o"][c * HSL : (c + 1) * HSL]),
            "mask": mask,
            "wconst": wconst,
        }
        in_maps.append(m)
    return in_maps


def kernel(x, cos, sin, wq, wk, wv, wo, qn, kn):
    if "nc" not in _CACHE:
        _CACHE["nc"] = _build()
    nc = _CACHE["nc"]
    in_maps = _host_inputs(x, cos, sin, wq, wk, wv, wo, qn, kn)
    res = bass_utils.run_bass_kernel_spmd(nc, in_maps, core_ids=list(range(NC)))
    outs = [np.asarray(res.results[c]["out"]) for c in range(NC)]
    return np.concatenate(outs, axis=0).reshape(B, S, H).astype(np.float32)


# revision 46
# speedup vs baseline: 1.0752x; 1.0399x over previous
"""BitNet GQA attention layer on 8 TRN2 NeuronCores.

Sharding: token-parallel. B*S = 2048 tokens -> 256 per core (core c: batch
c//4, quarter c%4). Weights are split 8-way along the contraction dim for
quantization (exact global absmean via tiny AllReduces), then the ternary
integer weights are AllGathered in bf16 (three pipelined AGs: k+v first so
K/V projections start early, then q, then o). K/V are AllGathered within
each batch's 4-core group, hidden under Q-projection + Q-rope. All BitNet
matmuls run as exact integer arithmetic in bf16 (acts in [-128,127],
weights in {-1,0,1}) with fp32 PSUM accumulation; dequantization scales are
applied to the fp32 results.
"""

import sys

sys.path.insert(0, "/opt/trn_rl_repo")

import numpy as np
import ml_dtypes

import concourse.bass as bass
import concourse.mybir as mybir
import concourse.tile as tile
from concourse import bacc
from concourse import bass_utils
from concourse.masks import make_identity

F32 = mybir.dt.float32
BF16 = mybir.dt.bfloat16
FP8 = mybir.dt.float8e4
AX = mybir.AxisListType.X
OP = mybir.AluOpType
AF = mybir.ActivationFunctionType

B, S, H = 2, 1024, 2048
NH, NKV, HD = 16, 8, 128
NC = 8
T = (B * S) // NC  # 256 tokens per core
TB = T // 128  # 2 token tiles per core
HSL = H // NC  # 256 weight rows per core
EPS = 1e-6
RND = 12582912.0  # 1.5 * 2**23: fp32 add => round-to-nearest-even
INV_SQRT_HD = 1.0 / float(np.sqrt(HD))
KTILES = S // 128  # 8 key tiles per batch
GROUP = 4  # cores per batch

OQ, OK, OV, OO = H, NKV * HD, NKV * HD, H  # 2048, 1024, 1024, 2048
OW = {"q": OQ, "k": OK, "v": OV, "o": OO}
WNUMEL = {m: OW[m] * H for m in OW}
HI_N = H // 128  # 16 contraction tiles

_CACHE = {}


def _build():
    nc = bacc.Bacc("TRN2", target_bir_lowering=False, debug=False, num_devices=NC)

    x_sl = nc.dram_tensor("x_sl", [T, H], F32, kind="ExternalInput")
    cosq = nc.dram_tensor("cosq", [T, HD], F32, kind="ExternalInput")
    sinq = nc.dram_tensor("sinq", [T, HD], F32, kind="ExternalInput")
    cosk = nc.dram_tensor("cosk", [T, HD], F32, kind="ExternalInput")
    sink = nc.dram_tensor("sink", [T, HD], F32, kind="ExternalInput")
    w_sl = {
        "q": nc.dram_tensor("wq_sl", [HSL, OQ], F32, kind="ExternalInput"),
        "k": nc.dram_tensor("wk_sl", [HSL, OK], F32, kind="ExternalInput"),
        "v": nc.dram_tensor("wv_sl", [HSL, OV], F32, kind="ExternalInput"),
        "o": nc.dram_tensor("wo_sl", [HSL, OO], F32, kind="ExternalInput"),
    }
    mask_in = nc.dram_tensor("mask", [128, KTILES + TB, T], BF16, kind="ExternalInput")
    # cols 0-3: numel for k,v,q,o ; cols 4-7: 1/numel for k,v,q,o
    wconst = nc.dram_tensor("wconst", [1, 8], F32, kind="ExternalInput")
    out = nc.dram_tensor("out", [T, H], F32, kind="ExternalOutput")

    with tile.TileContext(nc) as tc:
        _build_body(nc, tc, x_sl, cosq, sinq, cosk, sink, w_sl, mask_in, wconst, out)

    nc.compile()
    return nc


def _build_body(nc, tc, x_sl, cosq, sinq, cosk, sink, w_sl, mask_in, wconst, out):
    sync = nc.sync

    with (
        tc.tile_pool(name="dram", bufs=1, space="DRAM") as dram,
        tc.tile_pool(name="const", bufs=1) as constp,
        tc.tile_pool(name="vecs", bufs=1) as vecs,
        tc.tile_pool(name="persist", bufs=1) as persist,
        tc.tile_pool(name="ptrans", bufs=2, space="PSUM") as ptrans,
    ):
        # ---- DRAM bounce buffers for collectives ----
        wag_k = dram.tile([HSL, OK], FP8)
        wint_k = dram.tile([H, OK], FP8, addr_space="Shared")
        wag_v = dram.tile([HSL, OV], FP8)
        wint_v = dram.tile([H, OV], FP8, addr_space="Shared")
        wag_q = dram.tile([HSL, OQ], FP8)
        wint_q = dram.tile([H, OQ], FP8, addr_space="Shared")
        wag_o = dram.tile([HSL, OO], FP8)
        wint_o = dram.tile([H, OO], FP8, addr_space="Shared")
        ar1_in = dram.tile([1, 8], F32)
        ar1_out = dram.tile([1, 8], F32, addr_space="Shared")
        ar2_in = dram.tile([1, 8], F32)
        ar2_out = dram.tile([1, 8], F32, addr_space="Shared")
        k_in = dram.tile([128, NKV * T], BF16)
        k_out = dram.tile([512, NKV * T], BF16)
        v_in = dram.tile([128, TB * OV], BF16)
        v_out = dram.tile([512, TB * OV], BF16)

        # ---- constants ----
        ident = constp.tile([128, 128], F32)
        make_identity(nc, ident)
        ones1 = constp.tile([1, 128], F32)
        nc.vector.memset(ones1, 1.0)
        onescol = constp.tile([128, 1], F32)
        nc.vector.memset(onescol, 1.0)
        wconst_sb = constp.tile([1, 8], F32)
        sync.dma_start(wconst_sb, wconst.ap())
        negrnd = constp.tile([128, 1], F32)
        nc.vector.memset(negrnd, -RND)
        epsb = constp.tile([128, 1], F32)
        nc.vector.memset(epsb, EPS)
        cs = {}
        for nm, t in (("cq", cosq), ("sq", sinq), ("ck", cosk), ("sk", sink)):
            c = constp.tile([128, TB, HD], F32, name=f"cs_{nm}")
            sync.dma_start(c, t.ap().rearrange("(a p) d -> p a d", p=128))
            cs[nm] = c
        # persistent activations
        xqT = persist.tile([128, HI_N, T], BF16)  # [h%128, h//128, t]

        # ====== Phase W: weight scales + quantize + pipelined allgathers ======
        def w_sums_group(mats, psmall, ar_in, ar_out, wraws, label):
            """Load slices of `mats`, abs-sum, kick the AllReduce."""
            wab = {}
            for m in mats:
                for pt in range(2):
                    wr = wraws[m].tile([128, OW[m]], F32, name=f"wr_{m}{pt}")
                    sync.dma_start(wr, w_sl[m].ap()[pt * 128 : (pt + 1) * 128, :])
                    wab[(m, pt)] = wr
            red0 = vecs.tile([128, 2], F32, name=f"red0_{label}")
            red1 = vecs.tile([128, 2], F32, name=f"red1_{label}")
            for mi, m in enumerate(mats):
                for pt, red in ((0, red0), (1, red1)):
                    nc.vector.tensor_reduce(
                        red[:, mi : mi + 1], wab[(m, pt)], axis=AX, op=OP.add,
                        apply_absolute_value=True,
                    )
            redc = vecs.tile([128, 2], F32, name=f"redc_{label}")
            nc.vector.tensor_add(redc, red0, red1)
            ps = psmall.tile([1, 2], F32, name=f"ps_{label}", tag="psm")
            nc.tensor.matmul(ps, onescol, redc, start=True, stop=True)
            sums = vecs.tile([1, 8], F32, name=f"sums_{label}")
            nc.vector.memset(sums, 0.0)
            nc.scalar.copy(sums[:, 0:2], ps)
            nc.scalar.dma_start(ar_in, sums)
            nc.gpsimd.collective_compute(
                "AllReduce", OP.add, replica_groups=[list(range(NC))],
                ins=[ar_in.opt()], outs=[ar_out.opt()],
            )
            return wab

        def w_scales_group(mats, psmall, ar_out, label):
            """Read back the AllReduce and build the [128,4] scale tile."""
            g = vecs.tile([1, 8], F32, name=f"g_{label}")
            nc.scalar.dma_start(g, ar_out)
            r2 = vecs.tile([1, 2], F32, name=f"r2_{label}")
            nc.vector.reciprocal(r2, g[:, 0:2])
            sw4 = vecs.tile([1, 4], F32, name=f"sw4_{label}")
            ncol = {("k", "v"): (0, 2), ("q", "o"): (2, 4)}[tuple(mats)]
            nc.vector.tensor_mul(sw4[:, 0:2], r2, wconst_sb[:, ncol[0] : ncol[1]])
            nc.vector.tensor_mul(
                sw4[:, 2:4], g[:, 0:2], wconst_sb[:, 4 + ncol[0] : 4 + ncol[1]]
            )
            pb = psmall.tile([128, 4], F32, name=f"pb_{label}", tag="psm")
            nc.tensor.matmul(pb, ones1, sw4, start=True, stop=True)
            sb = vecs.tile([128, 4], F32, name=f"sb_{label}")
            nc.scalar.copy(sb, pb)
            return sb

        def w_quant(wab, m, mi, sb, wtmp, wq8, dst, col0):
            for pt in range(2):
                wr = wab[(m, pt)]
                tmp = wtmp.tile([128, OW[m]], F32, tag="wtmp")
                nc.vector.tensor_scalar(
                    tmp, wr, sb[:, mi : mi + 1], RND, op0=OP.mult, op1=OP.add
                )
                nc.vector.tensor_scalar(
                    tmp, tmp, -RND, 1.0, op0=OP.add, op1=OP.min
                )
                wi = wq8.tile([128, OW[m]], FP8, tag="wi")
                nc.vector.tensor_scalar(wi, tmp, -1.0, None, op0=OP.max)
                nc.scalar.dma_start(
                    dst[pt * 128 : (pt + 1) * 128, col0 : col0 + OW[m]], wi
                )

        rswb = {}
        with (
            tc.tile_pool(name="wraw_q", bufs=1) as wraw_q,
            tc.tile_pool(name="wraw_k", bufs=1) as wraw_k,
            tc.tile_pool(name="wraw_v", bufs=1) as wraw_v,
            tc.tile_pool(name="wraw_o", bufs=1) as wraw_o,
            tc.tile_pool(name="wtmp", bufs=2) as wtmp,
            tc.tile_pool(name="wq8", bufs=2) as wq8,
            tc.tile_pool(name="psmall", bufs=2, space="PSUM") as psmall,
        ):
            wraws = {"q": wraw_q, "k": wraw_k, "v": wraw_v, "o": wraw_o}
            wab_kv = w_sums_group(("k", "v"), psmall, ar1_in, ar1_out, wraws, "kv")
            # x loads right behind wk/wv so the x pipeline fills the AR window
            xs_t = []
            with tc.tile_pool(name="xraw", bufs=2) as xraw:
                for tb in range(TB):
                    xs = xraw.tile([128, H], F32, tag="xs", name=f"xs{tb}")
                    sync.dma_start(xs, x_sl.ap()[tb * 128 : (tb + 1) * 128, :])
                    xs_t.append(xs)
                wab_qo = w_sums_group(
                    ("q", "o"), psmall, ar2_in, ar2_out, wraws, "qo"
                )

                # ====== Phase X: act_quant(x) + transpose ======
                dqx = []
                for tb in range(TB):
                    xs = xs_t[tb]
                    axm = vecs.tile([128, 1], F32, name=f"axm{tb}")
                    nc.vector.tensor_reduce(
                        axm, xs, axis=AX, op=OP.max, apply_absolute_value=True
                    )
                    rsx = vecs.tile([128, 1], F32, name=f"rsx{tb}")
                    nc.vector.reciprocal(rsx, axm)
                    sxq = vecs.tile([128, 1], F32, name=f"sxq{tb}")
                    nc.vector.tensor_scalar_mul(sxq, rsx, 127.0)
                    dq = vecs.tile([128, 1], F32, name=f"dqx{tb}")
                    nc.vector.tensor_scalar_mul(dq, axm, 1.0 / 127.0)
                    dqx.append(dq)
                    nc.vector.tensor_scalar(
                        xs, xs, sxq, RND, op0=OP.mult, op1=OP.add
                    )
                    for hg in range(0, HI_N, 4):
                        pt4 = ptrans.tile([128, 4, 128], F32, tag="ptr")
                        for i in range(4):
                            hi = hg + i
                            nc.tensor.transpose(
                                pt4[:, i, :], xs[:, hi * 128 : (hi + 1) * 128], ident
                            )
                        nc.scalar.activation(
                            xqT[:, hg : hg + 4, tb * 128 : (tb + 1) * 128],
                            pt4, AF.Identity, bias=negrnd,
                        )

            sb_kv = w_scales_group(("k", "v"), psmall, ar1_out, "kv")
            w_quant(wab_kv, "k", 0, sb_kv, wtmp, wq8, wag_k, 0)
            nc.gpsimd.collective_compute(
                "AllGather", OP.bypass, replica_groups=[list(range(NC))],
                ins=[wag_k.opt()], outs=[wint_k.opt()],
            )
            w_quant(wab_kv, "v", 1, sb_kv, wtmp, wq8, wag_v, 0)
            nc.gpsimd.collective_compute(
                "AllGather", OP.bypass, replica_groups=[list(range(NC))],
                ins=[wag_v.opt()], outs=[wint_v.opt()],
            )
            rswb["k"] = sb_kv[:, 2:3]
            rswb["v"] = sb_kv[:, 3:4]
            sb_qo = w_scales_group(("q", "o"), psmall, ar2_out, "qo")
            w_quant(wab_qo, "q", 0, sb_qo, wtmp, wq8, wag_q, 0)
            nc.gpsimd.collective_compute(
                "AllGather", OP.bypass, replica_groups=[list(range(NC))],
                ins=[wag_q.opt()], outs=[wint_q.opt()],
            )
            w_quant(wab_qo, "o", 1, sb_qo, wtmp, wq8, wag_o, 0)
            rswb["q"] = sb_qo[:, 2:3]
            rswb["o"] = sb_qo[:, 3:4]

        # dequant vectors (absmax/127 * 1/s_w)
        dqv = {}
        for m in ("q", "k", "v", "o"):
            for tb in range(TB):
                d = vecs.tile([128, 1], F32, name=f"dqv_{m}{tb}")
                nc.vector.tensor_mul(d, dqx[tb], rswb[m])
                dqv[(m, tb)] = d

        q_sb = persist.tile([128, TB, OQ], F32, tag="qsb")
        k_sb = persist.tile([128, TB, OK], F32)
        v_loc = persist.tile([128, TB, NKV, 130], BF16)
        nc.vector.memset(v_loc, 1.0)
        qT = persist.tile([128, NH, T], BF16)  # [d, head, t]
        kT = persist.tile([128, NKV, T], BF16, tag="t8", padded_shape=[128, HI_N, T])

        def proj_load(wint_src, o_w, m, wpool):
            """Stream the fp8 weight matrix in 4 hi-chunks (pipelined)."""
            src3 = wint_src.rearrange("(hi p) o -> p hi o", p=128)
            chunks = []
            for cg in range(4):
                wst = wpool.tile(
                    [128, 4, o_w], FP8, tag="wst",
                    padded_shape=[128, 4, OQ], name=f"wst_{m}{cg}",
                )
                sync.dma_start(wst, src3[:, cg * 4 : (cg + 1) * 4, :])
                chunks.append(wst)
            return chunks

        def proj_tb(chunks, col0, o_w, m, tb, dst_fn, ppool):
            """dequant(xqT.T @ w_int) for one token tile over all o-chunks."""
            for oc in range(o_w // 512):
                pp = ppool.tile([128, 512], F32, tag="pp")
                for hi in range(HI_N):
                    nc.tensor.matmul(
                        pp,
                        xqT[:, hi, tb * 128 : (tb + 1) * 128],
                        chunks[hi // 4][
                            :, hi % 4, col0 + oc * 512 : col0 + (oc + 1) * 512
                        ],
                        start=(hi == 0),
                        stop=(hi == HI_N - 1),
                    )
                nc.vector.tensor_scalar(
                    dst_fn(tb, oc), pp, dqv[(m, tb)], None, op0=OP.mult
                )

        def rope_batch(src_sb, tb, nh, cosn, sinn, dstT, ropep, label):
            w = nh * 128
            blk = src_sb[:, tb, :]  # [128, w] f32
            sq = ropep.tile([128, w], F32, tag="unf", padded_shape=[128, NH * 128])
            nc.scalar.activation(sq, blk, AF.Square)
            ms = vecs.tile([128, nh], F32, name=f"ms_{label}{tb}")
            nc.vector.tensor_reduce(
                ms, sq.rearrange("p (h d) -> p h d", h=nh), axis=AX, op=OP.add
            )
            rms = vecs.tile([128, nh], F32, name=f"rms_{label}{tb}")
            nc.scalar.activation(rms, ms, AF.Sqrt, scale=1.0 / HD, bias=epsb)
            rn = vecs.tile([128, nh], F32, name=f"rn_{label}{tb}")
            nc.vector.reciprocal(rn, rms)
            rnb = rn.to_broadcast([128, nh, 128])
            blk3 = blk.rearrange("p (h d) -> p h d", h=nh)
            un2 = ropep.tile(
                [128, nh * 128], F32, tag="unf", padded_shape=[128, NH * 128],
                name="un2",
            )
            un = un2.rearrange("p (h d) -> p h d", h=nh)
            nc.vector.tensor_mul(un, blk3, rnb)
            cosb = (
                cs[cosn][:, tb, :]
                .rearrange("p (one d) -> p one d", one=1)
                .to_broadcast([128, nh, 128])
            )
            sinb = (
                cs[sinn][:, tb, :]
                .rearrange("p (one d) -> p one d", one=1)
                .to_broadcast([128, nh, 128])
            )
            ra = ropep.tile([128, nh, 128], F32, tag="ra", padded_shape=[128, NH, 128])
            nc.vector.tensor_mul(ra, un, cosb)
            rb = ropep.tile([128, nh, 128], F32, tag="rb", padded_shape=[128, NH, 128])
            nc.vector.tensor_mul(rb[:, :, 0:64], un[:, :, 64:128], sinb[:, :, 0:64])
            nc.vector.tensor_mul(rb[:, :, 64:128], un[:, :, 0:64], sinb[:, :, 64:128])
            nc.vector.tensor_add(ra, ra, rb)
            for hg in range(0, nh, 4):
                pt4 = ptrans.tile([128, 4, 128], F32, tag="ptr")
                for i in range(4):
                    nc.tensor.transpose(pt4[:, i, :], ra[:, hg + i, :], ident)
                nc.scalar.activation(
                    dstT[:, hg : hg + 4, tb * 128 : (tb + 1) * 128], pt4, AF.Copy
                )

        # ====== K/V projections + K rope + KV allgather ======
        with (
            tc.tile_pool(name="wmm1", bufs=8) as wmm1,
            tc.tile_pool(name="pproj1", bufs=3, space="PSUM") as pproj1,
            tc.tile_pool(name="ropek", bufs=1) as ropek,
        ):
            wst_k = proj_load(wint_k, OK, "k", wmm1)
            for tb in range(TB):
                proj_tb(wst_k, 0, OK, "k", tb,
                        lambda tb, oc: k_sb[:, tb, oc * 512 : (oc + 1) * 512],
                        pproj1)
                rope_batch(k_sb, tb, NKV, "ck", "sk", kT, ropek, "k")
            sync.dma_start(
                k_in.rearrange("p (hk t) -> p hk t", hk=NKV), kT
            )
            nc.gpsimd.collective_compute(
                "AllGather", OP.bypass,
                replica_groups=[[0, 1, 2, 3], [4, 5, 6, 7]],
                ins=[k_in.opt()], outs=[k_out.opt()],
            )
            wst_v = proj_load(wint_v, OV, "v", wmm1)
            for tb in range(TB):
                proj_tb(wst_v, 0, OV, "v", tb,
                        lambda tb, oc: v_loc[:, tb, oc * 4 : (oc + 1) * 4, 0:128],
                        pproj1)
            sync.dma_start(
                v_in.rearrange("p (a hk d) -> p a hk d", a=TB, hk=NKV),
                v_loc[:, :, :, 0:128],
            )
            nc.gpsimd.collective_compute(
                "AllGather", OP.bypass,
                replica_groups=[[0, 1, 2, 3], [4, 5, 6, 7]],
                ins=[v_in.opt()], outs=[v_out.opt()],
            )

            # ====== Q projection + Q rope (overlap the KV allgathers) ======
            wst_q = proj_load(wint_q, OQ, "q", wmm1)
            for tb in range(TB):
                proj_tb(wst_q, 0, OQ, "q", tb,
                        lambda tb, oc: q_sb[:, tb, oc * 512 : (oc + 1) * 512],
                        pproj1)
                rope_batch(q_sb, tb, NH, "cq", "sq", qT, ropek, "q")
            nc.gpsimd.collective_compute(
                "AllGather", OP.bypass, replica_groups=[list(range(NC))],
                ins=[wag_o.opt()], outs=[wint_o.opt()],
            )

        mask_sb = persist.tile([128, KTILES + TB, T], BF16)
        sync.dma_start(mask_sb, mask_in.ap())
        attn = persist.tile([128, TB, H], F32, tag="qsb")  # reuse q_sb slot
        attn_loc = persist.tile([128, TB, NH, 132], F32)

        # prefetch o_proj weights under the attention phase
        wmm2 = tc.tile_pool(name="wmm2", bufs=1).__enter__()
        src3o = wint_o.rearrange("(hi p) o -> p hi o", p=128)
        wsto = wmm2.tile([128, HI_N, OO], FP8, tag="wst2")
        sync.dma_start(wsto, src3o)

        # ====== attention ======
        with (
            tc.tile_pool(name="pscore", bufs=2, space="PSUM") as pscore,
            tc.tile_pool(name="ppv", bufs=2, space="PSUM") as ppv,
            tc.tile_pool(name="pexp", bufs=4) as pexp,
        ):
            # local part: own K/V tiles (diagonal blocks) - no collective dep
            for h in range(NH):
                hk = h // 2
                pel = pexp.tile([128, TB, T], BF16, tag="pel")
                st = pscore.tile([128, 2, T], F32, tag="st")
                for a in range(TB):
                    nc.tensor.matmul(
                        st[:, a, :], kT[:, hk, a * 128 : (a + 1) * 128], qT[:, h, :],
                        start=True, stop=True,
                    )
                nc.scalar.activation(pel, st, AF.Exp, scale=INV_SQRT_HD)
                nc.vector.tensor_mul(pel, pel, mask_sb[:, KTILES : KTILES + TB, :])
                for tb in range(TB):
                    po = ppv.tile([128, 132], F32, tag="po", padded_shape=[128, 132])
                    for a in range(TB):
                        nc.tensor.matmul(
                            po[:, 0:129],
                            pel[:, a, tb * 128 : (tb + 1) * 128],
                            v_loc[:, a, hk, 0:129],
                            start=(a == 0),
                            stop=(a == TB - 1),
                        )
                    nc.vector.tensor_copy(attn_loc[:, tb, h, 0:129], po[:, 0:129])

            # gather readback
            kT_all = persist.tile([128, NKV, KTILES, 128], BF16)
            v_all = persist.tile([128, KTILES, NKV, 130], BF16)
            nc.vector.memset(v_all, 1.0)
            for cb in range(GROUP):
                # kT part: k_out row = 128*cb + d ; col = hk*256 + a*128 + t
                src_k = k_out[cb * 128 : (cb + 1) * 128, :].rearrange(
                    "d (hk t) -> d hk t", hk=NKV
                )
                sync.dma_start(kT_all[:, :, 2 * cb : 2 * cb + 2, :], src_k)
            for cb in range(GROUP):
                # v part: v_out row = 128*cb + p ; col = a*1024 + hk*128 + d
                src_v = v_out[cb * 128 : (cb + 1) * 128, :].rearrange(
                    "p (a hk d) -> p a hk d", a=TB, hk=NKV
                )
                sync.dma_start(v_all[:, 2 * cb : 2 * cb + 2, :, 0:128], src_v)

            # remote part: strictly-below-diagonal tiles from the allgather
            for h in range(NH):
                hk = h // 2
                pe = pexp.tile([128, KTILES, T], BF16, tag="pe")
                for jp in range(KTILES // 2):
                    st = pscore.tile([128, 2, T], F32, tag="st")
                    for i in range(2):
                        nc.tensor.matmul(
                            st[:, i, :], kT_all[:, hk, 2 * jp + i, :], qT[:, h, :],
                            start=True, stop=True,
                        )
                    nc.scalar.activation(
                        pe[:, 2 * jp : 2 * jp + 2, :], st, AF.Exp, scale=INV_SQRT_HD
                    )
                nc.vector.tensor_mul(pe, pe, mask_sb[:, 0:KTILES, :])
                for tb in range(TB):
                    po = ppv.tile([128, 132], F32, tag="po", padded_shape=[128, 132])
                    for j in range(KTILES):
                        nc.tensor.matmul(
                            po[:, 0:129],
                            pe[:, j, tb * 128 : (tb + 1) * 128],
                            v_all[:, j, hk, 0:129],
                            start=(j == 0),
                            stop=(j == KTILES - 1),
                        )
                    cmb = pexp.tile([128, 132], F32, tag="cmb")
                    nc.vector.tensor_add(
                        cmb[:, 0:129], po[:, 0:129], attn_loc[:, tb, h, 0:129]
                    )
                    rden = vecs.tile([128, 1], F32, name=f"rden{h}_{tb}")
                    nc.vector.reciprocal(rden, cmb[:, 128:129])
                    nc.vector.tensor_scalar(
                        attn[:, tb, h * 128 : (h + 1) * 128],
                        cmb[:, 0:128], rden, None, op0=OP.mult,
                    )

        # ====== act_quant(attn) + o_proj ======
        with (
            tc.tile_pool(name="oq", bufs=2) as oq,
            tc.tile_pool(name="pproj2", bufs=3, space="PSUM") as pproj2,
            tc.tile_pool(name="osb", bufs=2) as osb,
        ):
            aT = persist.tile([128, HI_N, T], BF16, tag="t8")
            dqo = []
            for tb in range(TB):
                axm = vecs.tile([128, 1], F32, name=f"oaxm{tb}")
                nc.vector.tensor_reduce(
                    axm, attn[:, tb, :], axis=AX, op=OP.max,
                    apply_absolute_value=True,
                )
                rsx = vecs.tile([128, 1], F32, name=f"orsx{tb}")
                nc.vector.reciprocal(rsx, axm)
                sxq = vecs.tile([128, 1], F32, name=f"osxq{tb}")
                nc.vector.tensor_scalar_mul(sxq, rsx, 127.0)
                dq = vecs.tile([128, 1], F32, name=f"odqx{tb}")
                nc.vector.tensor_scalar_mul(dq, axm, 1.0 / 127.0)
                d2 = vecs.tile([128, 1], F32, name=f"odq2{tb}")
                nc.vector.tensor_mul(d2, dq, rswb["o"])
                dqo.append(d2)
                ar = oq.tile([128, H], F32, tag="ar")
                nc.vector.tensor_scalar(
                    ar, attn[:, tb, :], sxq, RND, op0=OP.mult, op1=OP.add
                )
                for hg in range(0, HI_N, 4):
                    pt4 = ptrans.tile([128, 4, 128], F32, tag="ptr")
                    for i in range(4):
                        hi = hg + i
                        nc.tensor.transpose(
                            pt4[:, i, :], ar[:, hi * 128 : (hi + 1) * 128], ident
                        )
                    nc.scalar.activation(
                        aT[:, hg : hg + 4, tb * 128 : (tb + 1) * 128],
                        pt4, AF.Identity, bias=negrnd,
                    )

            for tb in range(TB):
                for oc in range(OO // 512):
                    pp = pproj2.tile([128, 512], F32, tag="pp2")
                    for hi in range(HI_N):
                        nc.tensor.matmul(
                            pp,
                            aT[:, hi, tb * 128 : (tb + 1) * 128],
                            wsto[:, hi, oc * 512 : (oc + 1) * 512],
                            start=(hi == 0),
                            stop=(hi == HI_N - 1),
                        )
                    ot = osb.tile([128, 512], F32, tag="ot")
                    nc.vector.tensor_scalar(ot, pp, dqo[tb], None, op0=OP.mult)
                    sync.dma_start(
                        out.ap()[
                            tb * 128 : (tb + 1) * 128, oc * 512 : (oc + 1) * 512
                        ],
                        ot,
                    )


def _host_inputs(x, cos, sin, wq, wk, wv, wo, qn, kn):
    """Build the 8 per-core input maps (pure slicing / layout transforms)."""
    x2 = np.asarray(x, np.float32).reshape(B * S, H)
    cos = np.asarray(cos, np.float32)
    sin = np.asarray(sin, np.float32)
    qn = np.asarray(qn, np.float32)
    kn = np.asarray(kn, np.float32)
    # fold qk-norm weights into rope tables (exact identity when qn=kn=1)
    qn_rot = np.concatenate([qn[HD // 2 :], qn[: HD // 2]])
    kn_rot = np.concatenate([kn[HD // 2 :], kn[: HD // 2]])
    sgn = np.concatenate(
        [-np.ones(HD // 2, np.float32), np.ones(HD // 2, np.float32)]
    )
    cosq_t = cos * qn[None, :]
    sinq_t = sin * (qn_rot * sgn)[None, :]
    cosk_t = cos * kn[None, :]
    sink_t = sin * (kn_rot * sgn)[None, :]

    wt = {
        "q": np.asarray(wq, np.float32).T,  # [H, OQ]
        "k": np.asarray(wk, np.float32).T,
        "v": np.asarray(wv, np.float32).T,
        "o": np.asarray(wo, np.float32).T,  # [H(=in), OO]
    }
    worder = ("k", "v", "q", "o")
    wconst = np.concatenate(
        [
            np.array([WNUMEL[m] for m in worder], np.float32),
            np.array([1.0 / WNUMEL[m] for m in worder], np.float32),
        ]
    ).reshape(1, 8)

    in_maps = []
    for c in range(NC):
        qt = c % GROUP
        t0 = qt * T
        # strict mask [p, j, f]: key (128j+p) fully below this core's window
        p = np.arange(128)[:, None, None]
        j = np.arange(KTILES)[None, :, None]
        f = np.arange(T)[None, None, :]
        strict = ((128 * j + p) < t0) & (f >= 0)
        # diagonal masks for the two local key tiles
        a = np.arange(TB)[None, :, None]
        diag = (128 * a + p) <= f
        mask = np.concatenate([strict, diag], axis=1).astype(ml_dtypes.bfloat16)
        m = {
            "x_sl": np.ascontiguousarray(x2[c * T : (c + 1) * T]),
            "cosq": np.ascontiguousarray(cosq_t[t0 : t0 + T]),
            "sinq": np.ascontiguousarray(sinq_t[t0 : t0 + T]),
            "cosk": np.ascontiguousarray(cosk_t[t0 : t0 + T]),
            "sink": np.ascontiguousarray(sink_t[t0 : t0 + T]),
            "wq_sl": np.ascontiguousarray(wt["q"][c * HSL : (c + 1) * HSL]),
            "wk_sl": np.ascontiguousarray(wt["k"][c * HSL : (c + 1) * HSL]),
            "wv_sl": np.ascontiguousarray(wt["v"][c * HSL : (c + 1) * HSL]),
            "wo_sl": np.ascontiguousarray(wt["o"][c * HSL : (c + 1) * HSL]),
            "mask": mask,
            "wconst": wconst,
        }
        in_maps.append(m)
    return in_maps


def kernel(x, cos, sin, wq, wk, wv, wo, qn, kn):
    if "nc" not in _CACHE:
        _CACHE["nc"] = _build()
    nc = _CACHE["nc"]
    in_maps = _host_inputs(x, cos, sin, wq, wk, wv, wo, qn, kn)
    res = bass_utils.run_bass_kernel_spmd(nc, in_maps, core_ids=list(range(NC)))
    outs = [np.asarray(res.results[c]["out"]) for c in range(NC)]
    return np.concatenate(outs, axis=0).reshape(B, S, H).astype(np.float32)
